# revision 16
# baseline (speedup 1.0000x reference)
"""Multi-head attention (B=4, S=2048, D=1024, H=16) on 8 Trainium2 cores.

Sharding: core c handles batch b = c//2 and head-group hg = c%2 (8 of the 16
heads, 512 of the 1024 projection dims).  Host sums the two head-group
partials per batch (the "all-reduce after w_o") and adds bv@Wo + bo.

v2 design (vs the 425us baseline):
  * Pair-blocks: per (qc, pr) block both heads' scores are computed with
    64-row PE tiling (K=64 row tiles run concurrently -> 2x score matmul
    throughput; probe-measured 110ns/MM vs 216 standard).
  * Q is pre-scaled by log2(e)/8 on the host (folded into Wq/bq) so scores
    are in the exp2 domain.  exp for head-even tiles runs on ACT
    (exp(y*ln2)); head-odd tiles mostly run on the DVE as a one-instruction
    Schraudolph exp2 (tensor_scalar mult+add -> int16 bits == bf16 exp2
    approximation, ~3.3% max elementwise, cancels through softmax to ~1e-2
    final).  This splits the 293us exp load across two engines.
  * PSUM: SA + SB score tiles + two AV accumulators = exactly 8 banks.
    The l (softmax denominator) rides the AV matmul as V's 65th column.
  * Phase A emits K, V, Q(pair0) only; Q(pair 1-3) projections trail at
    qc0 block boundaries, C chunks of qc0 at qc1 boundaries, C of qc1 in
    the drain.  1/l uses reciprocal_approx_fast (5x the DVE reciprocal).

All matmuls are bf16 with fp32 PSUM accumulation.
"""

import os
import numpy as np

B, S, D = 4, 2048, 1024
H, DK = 16, 64
P = 128
NCORES = 8
HPC = H // 2            # heads per core
PROJ = HPC * DK         # 512 projection dims per core
NDM = D // P            # 8 d_model chunks
NPC = PROJ // P         # 4 head-pair chunks
NSC = S // 512          # 4 seq chunks of 512
NSO = S // P            # 16 seq chunks of 128
NKC = S // P            # 16 key chunks of 128

L2E = float(np.log2(np.e))
LN2 = float(np.log(2.0))
C_SCH = 16250.5
# Every kc runs one exp tile on ACT and one on the DVE (Schraudolph), so the
# two engines stream in parallel.  kc in DVE_KCS -> head-odd tile on DVE;
# else head-even on DVE.  Alternating keeps the approximation error split
# evenly across both heads.  f_schraudolph = 0.5 structurally.
DVE_KCS = frozenset(range(0, NKC, 2))

_cache = {}


def _build():
    import concourse.bass as bass
    import concourse.bacc as bacc
    import concourse.mybir as mybir
    import concourse.tile as tile
    from contextlib import ExitStack

    f32 = mybir.dt.float32
    bf16 = mybir.dt.bfloat16
    i16 = mybir.dt.int16
    AF = mybir.ActivationFunctionType
    MUL = mybir.AluOpType.mult
    ADD = mybir.AluOpType.add

    nc = bacc.Bacc("TRN2", target_bir_lowering=False, debug=False,
                   num_devices=NCORES)

    # Activations/weights arrive pre-permuted from the host so every DMA is
    # a dense per-partition burst (8KB lines) instead of 1KB strided lines.
    qT = nc.dram_tensor("qT", [NSC, P, NDM, 512], bf16, kind="ExternalInput").ap()
    kT = nc.dram_tensor("kT", [NSC, P, NDM, 512], bf16, kind="ExternalInput").ap()
    vT = nc.dram_tensor("vT", [NSC, P, NDM, 512], bf16, kind="ExternalInput").ap()
    wq = nc.dram_tensor("wq", [P, NDM, PROJ], bf16, kind="ExternalInput").ap()
    wk = nc.dram_tensor("wk", [P, NDM, PROJ], bf16, kind="ExternalInput").ap()
    wv = nc.dram_tensor("wv", [P, NDM, PROJ], bf16, kind="ExternalInput").ap()
    wo = nc.dram_tensor("wo", [P, NPC, D], bf16, kind="ExternalInput").ap()
    bq2 = nc.dram_tensor("bq2", [P, NPC], f32, kind="ExternalInput").ap()
    bk2 = nc.dram_tensor("bk2", [P, NPC], f32, kind="ExternalInput").ap()
    mk = nc.dram_tensor("mk", [P, NSO], f32, kind="ExternalInput").ap()
    sel = nc.dram_tensor("sel", [P, 128], bf16, kind="ExternalInput").ap()
    out = nc.dram_tensor("out", [S, D], f32, kind="ExternalOutput").ap()

    with tile.TileContext(nc) as tc, ExitStack() as ctx:
        cpool = ctx.enter_context(tc.tile_pool(name="const", bufs=1))
        sel_sb = cpool.tile([P, 128], bf16)
        nc.sync.dma_start(sel_sb[:], sel)
        bq_sb = cpool.tile([P, NPC], f32)
        nc.sync.dma_start(bq_sb[:], bq2)
        bk_sb = cpool.tile([P, NPC], f32)
        nc.sync.dma_start(bk_sb[:], bk2)
        mk_sb = cpool.tile([P, NSO], f32)
        nc.sync.dma_start(mk_sb[:], mk)
        # l values land in rows {0, 32}; other rows must stay finite for the
        # sel broadcast matmul.
        Lsb = cpool.tile([P, 1024], bf16)
        nc.gpsimd.memset(Lsb[:], 0.0)
        ones8 = cpool.tile([P, HPC], bf16)
        nc.gpsimd.memset(ones8[:], 1.0)

        respool = ctx.enter_context(tc.tile_pool(name="res", bufs=1))
        # Q^T/K^T pair-stacked: rows 0-63 = head 2*pr dims, 64-127 = head
        # 2*pr+1.  The K projection writes this layout directly (proj dims
        # are naturally pair-major), and the 64-row-tiled score matmuls take
        # lhsT/rhs from the matching partition halves.
        QT_sb = respool.tile([P, NPC, S], bf16)
        KT_sb = respool.tile([P, NPC, S], bf16)
        # V with an interleaved mask column per head: head h occupies cols
        # [h*65, h*65+64) and col h*65+64 == mask (the masked softmax
        # denominator rides the AV matmul as output partition 64).
        V_sb = respool.tile([P, NSO, HPC * (DK + 1)], bf16)
        for so in range(NSO):
            # mask columns filled on gpsimd so the DVE queue stays clear for
            # the K-projection bias adds.
            nc.gpsimd.tensor_scalar_mul(
                V_sb[:, so, :].rearrange("p (h w) -> p h w", w=DK + 1)[:, :, DK],
                ones8[:], mk_sb[:, so:so + 1])
        AT_sb = respool.tile([P, NPC, S], bf16)   # normalized A^T

        wopool = ctx.enter_context(tc.tile_pool(name="wo", bufs=1))
        wo_sb = wopool.tile([P, NPC, D], bf16)

        npool = ctx.enter_context(tc.tile_pool(name="norm", bufs=4))
        rcpool = ctx.enter_context(tc.tile_pool(name="rc", bufs=2))
        epA = ctx.enter_context(tc.tile_pool(name="expA", bufs=5))
        epB = ctx.enter_context(tc.tile_pool(name="expB", bufs=6))
        opool = ctx.enter_context(tc.tile_pool(name="ostage", bufs=4))

        # Weight + activation staging pools live for the whole kernel: wq is
        # needed for trailing Q projections inside phase B.
        wpool = ctx.enter_context(tc.tile_pool(name="w", bufs=2))
        apool = ctx.enter_context(tc.tile_pool(name="act", bufs=4))

        # ---------------- Phase A: K, V, Q(pair0) ----------------
        with ExitStack() as ctxA:
            psA = ctxA.enter_context(
                tc.tile_pool(name="psA", bufs=4, space="PSUM"))

            # K projection -> pair-packed KT_sb
            wk_sb = wpool.tile([P, NDM, PROJ], bf16, tag="w", name="wk_sb")
            nc.sync.dma_start(wk_sb[:], wk)
            for sc in range(NSC):
                a_sb = apool.tile([P, NDM, 512], bf16, tag="a", name="a_sb")
                nc.sync.dma_start(a_sb[:], kT[sc])
                for pc in range(NPC):
                    ps = psA.tile([P, 512], f32, tag="pp", name="psa")
                    for dc in range(NDM):
                        nc.tensor.matmul(
                            ps,
                            lhsT=wk_sb[:, dc, pc * P:(pc + 1) * P],
                            rhs=a_sb[:, dc, :],
                            start=(dc == 0), stop=(dc == NDM - 1))
                    nc.vector.tensor_scalar_add(
                        KT_sb[:, pc, sc * 512:(sc + 1) * 512], ps,
                        bk_sb[:, pc:pc + 1])

            # V projection (mask folded in)
            wv_sb = wpool.tile([P, NDM, PROJ], bf16, tag="w", name="wv_sb")
            nc.sync.dma_start(wv_sb[:], wv)
            for sc in range(NSC):
                a_sb = apool.tile([P, NDM, 512], bf16, tag="a", name="a_sb")
                nc.sync.dma_start(a_sb[:], vT[sc])
                for so4 in range(4):
                    so = sc * 4 + so4
                    ps = psA.tile([P, 512], f32, tag="pp", name="psa")
                    for dc in range(NDM):
                        nc.tensor.matmul(
                            ps,
                            lhsT=a_sb[:, dc, so4 * P:(so4 + 1) * P],
                            rhs=wv_sb[:, dc, :],
                            start=(dc == 0), stop=(dc == NDM - 1))
                    nc.vector.tensor_scalar_mul(
                        V_sb[:, so, :].rearrange(
                            "p (h w) -> p h w", w=DK + 1)[:, :, 0:DK],
                        ps.rearrange("p (h w) -> p h w", w=DK),
                        mk_sb[:, so:so + 1])

            # Q projection, pair 0 only; wq stays resident for pairs 1-3.
            wq_sb = wpool.tile([P, NDM, PROJ], bf16, tag="w", name="wq_sb")
            nc.sync.dma_start(wq_sb[:], wq)
            for sc in range(NSC):
                a_sb = apool.tile([P, NDM, 512], bf16, tag="a", name="a_sb")
                nc.sync.dma_start(a_sb[:], qT[sc])
                ps = psA.tile([P, 512], f32, tag="pp", name="psa")
                for dc in range(NDM):
                    nc.tensor.matmul(
                        ps,
                        lhsT=wq_sb[:, dc, 0:P],
                        rhs=a_sb[:, dc, :],
                        start=(dc == 0), stop=(dc == NDM - 1))
                nc.vector.tensor_scalar_add(
                    QT_sb[:, 0, sc * 512:(sc + 1) * 512], ps,
                    bq_sb[:, 0:1])
            nc.sync.dma_start(wo_sb[:], wo)

        # ---------------- Phase B ----------------
        psSA = ctx.enter_context(tc.tile_pool(name="psSA", bufs=1, space="PSUM"))
        psSB = ctx.enter_context(tc.tile_pool(name="psSB", bufs=1, space="PSUM"))
        psAcA = ctx.enter_context(tc.tile_pool(name="psAcA", bufs=1, space="PSUM"))
        psAcB = ctx.enter_context(tc.tile_pool(name="psAcB", bufs=1, space="PSUM"))

        qstage = {}

        def qprefetch(pc):
            """DMA the qT chunks for pair pc's trailing projection."""
            tiles = []
            for sc in range(NSC):
                a_sb = apool.tile([P, NDM, 512], bf16, tag="a", name="a_q")
                nc.sync.dma_start(a_sb[:], qT[sc])
                tiles.append(a_sb)
            qstage[pc] = tiles

        def emit_qproj(pc):
            """Trailing Q projection for pair pc (borrows psSA/psSB)."""
            tiles = qstage.pop(pc)
            for sc in range(NSC):
                a_sb = tiles[sc]
                pool = psSB if sc % 2 == 0 else psSA
                ps = pool.tile([P, 512], f32, tag="s", name="ps_q")
                for dc in range(NDM):
                    nc.tensor.matmul(
                        ps,
                        lhsT=wq_sb[:, dc, pc * P:(pc + 1) * P],
                        rhs=a_sb[:, dc, :],
                        start=(dc == 0), stop=(dc == NDM - 1))
                nc.vector.tensor_scalar_add(
                    QT_sb[:, pc, sc * 512:(sc + 1) * 512], ps,
                    bq_sb[:, pc:pc + 1])

        def emit_c_chunk(so, oc, pool):
            """Output projection chunk [128q, 512o] (borrows a score pool)."""
            ps = pool.tile([P, 512], f32, tag="s", name="ps_c")
            for pc in range(NPC):
                nc.tensor.matmul(
                    ps,
                    lhsT=AT_sb[:, pc, so * P:(so + 1) * P],
                    rhs=wo_sb[:, pc, oc * 512:(oc + 1) * 512],
                    start=(pc == 0), stop=(pc == NPC - 1))
            ost = opool.tile([P, 512], f32, tag="o", name="ost")
            nc.scalar.copy(ost, ps)
            nc.sync.dma_start(
                out[so * P:(so + 1) * P, oc * 512:(oc + 1) * 512], ost)

        def emit_scores_exp(pr, qc, kc):
            """Row-tiled pair scores + the pair's exp, one tile per engine.

            Emission is pinned so the two 64-row tiles of each sub stay
            adjacent (concurrent sub-array issue).  The DVE tile is
            processed in two N=512 halves so its score bank frees early.
            """
            sa = psSA.tile([P, 1024], f32, tag="s", name="sa")
            sb = psSB.tile([P, 1024], f32, tag="s", name="sb")
            ea = epA.tile([P, 1024], bf16, tag="e", name="ea")
            eb = epB.tile([P, 1024], bf16, tag="e", name="eb")
            if kc in DVE_KCS:
                act_t, act_ps, dve_t, dve_ps = ea, sa, eb, sb
            else:
                act_t, act_ps, dve_t, dve_ps = eb, sb, ea, sa
            for sub in range(2):
                ssl = slice(sub * 512, (sub + 1) * 512)
                cols = slice(qc * 1024 + sub * 512, qc * 1024 + (sub + 1) * 512)
                nc.tensor.matmul(
                    sa[:, ssl],
                    lhsT=KT_sb[0:64, pr, kc * P:(kc + 1) * P],
                    rhs=QT_sb[0:64, pr, cols],
                    start=True, stop=True)
                nc.tensor.matmul(
                    sb[:, ssl],
                    lhsT=KT_sb[64:128, pr, kc * P:(kc + 1) * P],
                    rhs=QT_sb[64:128, pr, cols],
                    start=True, stop=True)
                nc.vector.tensor_scalar(
                    dve_t[:, ssl].bitcast(i16), dve_ps[:, ssl],
                    128.0, C_SCH, MUL, ADD)
                if sub == 0:
                    tc.no_sync_barrier()
                else:
                    nc.scalar.activation(act_t, act_ps, AF.Exp, scale=LN2)
            return ea, eb

        def emit_av(acc, h, e, kc, start, stop):
            for sub in range(2):
                nc.tensor.matmul(
                    acc[0:DK + 1, sub * 512:(sub + 1) * 512],
                    lhsT=V_sb[:, kc, h * (DK + 1):(h + 1) * (DK + 1)],
                    rhs=e[:, sub * 512:(sub + 1) * 512],
                    start=start, stop=stop)

        def block_tails(pr, qc, accA, accB, esA, esB):
            """Last AVs, PSUM evacuation, l rows, 1/l, normalize -> AT_sb.

            Evacuation copies run on ACT (which has slack), the all-SBUF
            normalize multiplies on GPSIMD (idle otherwise), keeping the DVE
            free for the exp stream.  bc reuses accB's banks so both score
            banks are immediately available for boundary chunks.
            """
            emit_av(accA, 2 * pr + 0, esA[NKC - 2], NKC - 2, False, False)
            emit_av(accA, 2 * pr + 0, esA[NKC - 1], NKC - 1, False, True)
            emit_av(accB, 2 * pr + 1, esB[NKC - 2], NKC - 2, False, False)
            emit_av(accB, 2 * pr + 1, esB[NKC - 1], NKC - 1, False, True)
            nc.scalar.copy(Lsb[0:1, :], accA[DK:DK + 1, :])
            nc.scalar.copy(Lsb[32:33, :], accB[DK:DK + 1, :])
            atA = npool.tile([P, 1024], f32, tag="at", name="atA")
            nc.scalar.copy(atA[0:64, :], accA[0:64, :])
            atB = npool.tile([P, 1024], f32, tag="at", name="atB")
            nc.scalar.copy(atB[64:128, :], accB[0:64, :])
            # broadcast l to the heads' dim rows, reciprocal, normalize
            bc = psAcB.tile([P, 1024], f32, tag="av", name="bc")
            for sub in range(2):
                nc.tensor.matmul(
                    bc[:, sub * 512:(sub + 1) * 512],
                    lhsT=sel_sb[:],
                    rhs=Lsb[:, sub * 512:(sub + 1) * 512],
                    start=True, stop=True)
            rc = rcpool.tile([P, 1024], f32, tag="rc", name="rc")
            nc.vector.reciprocal_approx_fast(out=rc[:], in_=bc[:])
            nc.gpsimd.tensor_mul(
                AT_sb[0:64, pr, qc * 1024:(qc + 1) * 1024],
                atA[0:64, :], rc[0:64, :])
            nc.gpsimd.tensor_mul(
                AT_sb[64:128, pr, qc * 1024:(qc + 1) * 1024],
                atB[64:128, :], rc[64:128, :])

        blocks = [(qc, pr) for qc in range(2) for pr in range(NPC)]
        for bi, (qc, pr) in enumerate(blocks):
            esA = [None] * NKC
            esB = [None] * NKC
            accA = accB = None
            for kc in range(NKC):
                esA[kc], esB[kc] = emit_scores_exp(pr, qc, kc)
                if kc == 6 and qc == 0 and pr < 3:
                    qprefetch(pr + 1)
                # Pin the PE-mode groups: the 64-row-tiled score pair must
                # stay adjacent (concurrent sub-array issue) and the standard
                # AV matmuls must not interleave into it -- each tiling-mode
                # change drains the PE array.
                tc.no_sync_barrier()
                if kc == 2:
                    accA = psAcA.tile([P, 1024], f32, tag="av", name="accA")
                    accB = psAcB.tile([P, 1024], f32, tag="av", name="accB")
                    emit_av(accA, 2 * pr + 0, esA[0], 0, True, False)
                    emit_av(accB, 2 * pr + 1, esB[0], 0, True, False)
                elif kc > 2:
                    emit_av(accA, 2 * pr + 0, esA[kc - 2], kc - 2, False, False)
                    emit_av(accB, 2 * pr + 1, esB[kc - 2], kc - 2, False, False)
                if kc >= 2:
                    tc.no_sync_barrier()
            block_tails(pr, qc, accA, accB, esA, esB)

            # boundary work: trailing Q projections during qc0, qc0's output
            # projection during qc1 boundaries; qc1's C lands in the drain.
            tc.no_sync_barrier()
            if qc == 0 and pr < 3:
                emit_qproj(pr + 1)
            if bi >= 3:
                # 4 qc0 C-chunks at each of the last 5 boundaries handles
                # 16 chunks by the end of block (1, pr2); emit 4 per boundary
                # starting after block (0, pr3).
                cidx = (bi - 3) * 4
                if cidx < 16:
                    for j in range(cidx, cidx + 4):
                        so, oc = j // 2, j % 2
                        emit_c_chunk(so, oc, psSA if j % 2 == 0 else psSB)

        # drain: qc1's output projection
        tc.no_sync_barrier()
        for j in range(16):
            so, oc = 8 + j // 2, j % 2
            emit_c_chunk(so, oc, psSA if j % 2 == 0 else psSB)

    nc.compile()
    return nc


def _get_nc():
    if "nc" not in _cache:
        _cache["nc"] = _build()
    return _cache["nc"]


def make_in_maps(q, k, v, mask, Wq, bq, Wk, bk, Wv, bv, Wo, bo):
    """Host-side sharding: slice/transpose the full inputs per core."""
    import ml_dtypes
    f = np.float32
    bf = ml_dtypes.bfloat16
    q = np.asarray(q, dtype=f)
    k = np.asarray(k, dtype=f)
    v = np.asarray(v, dtype=f)
    Wq = np.asarray(Wq, dtype=f) * (L2E / 8.0)   # exp2-domain prescale
    Wk = np.asarray(Wk, dtype=f)
    Wv = np.asarray(Wv, dtype=f)
    Wo = np.asarray(Wo, dtype=f)
    bq = np.asarray(bq, dtype=f) * (L2E / 8.0)
    bk = np.asarray(bk, dtype=f)
    mask = np.asarray(mask)

    sel = np.zeros((P, 128), dtype=f)
    sel[0, 0:64] = 1.0
    sel[32, 64:128] = 1.0

    def act_perm(x):
        # [S, D] -> [NSC, P, NDM, 512]: chunk sc holds x^T[d, sc*512 + j]
        # with d = o*128 + p, laid out as one dense 8KB line per partition.
        return np.ascontiguousarray(
            x.reshape(NSC, 512, NDM, P).transpose(0, 3, 2, 1)).astype(bf)

    def w_perm(w):
        # [D, PROJ] -> [P, NDM, PROJ]
        return np.ascontiguousarray(
            w.reshape(NDM, P, -1).transpose(1, 0, 2)).astype(bf)

    in_maps = []
    for c in range(NCORES):
        b, hg = divmod(c, 2)
        cols = slice(hg * PROJ, (hg + 1) * PROJ)
        mvals = (mask[b, 0, 0, :] != 0).astype(f)            # [S]
        mk2 = np.ascontiguousarray(mvals.reshape(NSO, P).T)  # [P, NSO]
        in_maps.append({
            "qT": act_perm(q[b]),
            "kT": act_perm(k[b]),
            "vT": act_perm(v[b]),
            "wq": w_perm(Wq[:, cols]),
            "wk": w_perm(Wk[:, cols]),
            "wv": w_perm(Wv[:, cols]),
            "wo": np.ascontiguousarray(
                Wo[cols, :].reshape(NPC, P, D).transpose(1, 0, 2)).astype(bf),
            "bq2": np.ascontiguousarray(bq[cols].reshape(NPC, P).T),
            "bk2": np.ascontiguousarray(bk[cols].reshape(NPC, P).T),
            "mk": mk2,
            "sel": sel.astype(bf),
        })
    return in_maps


def combine_outputs(parts, Wv_bv_Wo_bo):
    """Sum the two head-group partials per batch, add bv @ Wo + bo."""
    bv, Wo, bo = Wv_bv_Wo_bo
    bo_eff = (np.asarray(bv, np.float32) @ np.asarray(Wo, np.float32)
              + np.asarray(bo, np.float32))
    out = np.empty((B, S, D), dtype=np.float32)
    for b in range(B):
        out[b] = parts[2 * b] + parts[2 * b + 1] + bo_eff
    return out


def _install_axon_ntff_hook():
    """The agent image's antenv lacks axon_hooks; synthesize it and register
    the ctypes NTFF profile hook from trn_boot so trace=True works."""
    import sys
    import types
    if "antenv.axon_hooks" in sys.modules:
        return
    try:
        from trn_agent_boot.trn_boot import _ntff_profile_via_ctypes
        hook = _ntff_profile_via_ctypes("/opt/axon/libaxon_pjrt.so")
    except Exception:
        hook = None
    mod = types.ModuleType("antenv.axon_hooks")
    mod._hook = hook
    mod.get_axon_ntff_profile_hook = lambda: mod._hook
    mod.set_axon_ntff_profile_hook = lambda h: setattr(mod, "_hook", h)
    sys.modules["antenv.axon_hooks"] = mod
    import concourse.bass_utils as bu
    bu.upload_artifacts = lambda tmpdir: str(tmpdir)


def kernel(q, k, v, mask, Wq, bq, Wk, bk, Wv, bv, Wo, bo):
    from concourse.bass_utils import run_bass_kernel_spmd

    nc = _get_nc()
    in_maps = make_in_maps(q, k, v, mask, Wq, bq, Wk, bk, Wv, bv, Wo, bo)
    trace = bool(int(os.environ.get("KERNEL_TRACE", "0")))
    if trace:
        try:
            _install_axon_ntff_hook()
        except Exception:
            trace = False
    try:
        res = run_bass_kernel_spmd(
            nc, in_maps, list(range(NCORES)), trace=trace,
            tmpdir=os.environ.get("KERNEL_TRACE_DIR") or None)
    except Exception:
        if not trace:
            raise
        res = run_bass_kernel_spmd(nc, in_maps, list(range(NCORES)), trace=False)
    _cache["last_result"] = res
    parts = [res.results[c]["out"] for c in range(NCORES)]
    return combine_outputs(parts, (bv, Wo, bo))


# revision 20
# speedup vs baseline: 1.2786x; 1.2786x over previous
"""Multi-head attention (B=4, S=2048, D=1024, H=16) on 8 Trainium2 cores.

Sharding: core c handles batch b = c//2 and head-group hg = c%2 (8 of the 16
heads, 512 of the 1024 projection dims).  Host sums the two head-group
partials per batch (the "all-reduce after w_o") and adds bv@Wo + bo.

v2 design (vs the 425us baseline):
  * Pair-blocks: per (qc, pr) block both heads' scores are computed with
    64-row PE tiling (K=64 row tiles run concurrently -> 2x score matmul
    throughput; probe-measured 110ns/MM vs 216 standard).
  * Q is pre-scaled by log2(e)/8 on the host (folded into Wq/bq) so scores
    are in the exp2 domain.  exp for head-even tiles runs on ACT
    (exp(y*ln2)); head-odd tiles mostly run on the DVE as a one-instruction
    Schraudolph exp2 (tensor_scalar mult+add -> int16 bits == bf16 exp2
    approximation, ~3.3% max elementwise, cancels through softmax to ~1e-2
    final).  This splits the 293us exp load across two engines.
  * PSUM: SA + SB score tiles + two AV accumulators = exactly 8 banks.
    The l (softmax denominator) rides the AV matmul as V's 65th column.
  * Phase A emits K, V, Q(pair0) only; Q(pair 1-3) projections trail at
    qc0 block boundaries, C chunks of qc0 at qc1 boundaries, C of qc1 in
    the drain.  1/l uses reciprocal_approx_fast (5x the DVE reciprocal).

All matmuls are bf16 with fp32 PSUM accumulation.
"""

import os
import numpy as np

B, S, D = 4, 2048, 1024
H, DK = 16, 64
P = 128
NCORES = 8
HPC = H // 2            # heads per core
PROJ = HPC * DK         # 512 projection dims per core
NDM = D // P            # 8 d_model chunks
NPC = PROJ // P         # 4 head-pair chunks
NSC = S // 512          # 4 seq chunks of 512
NSO = S // P            # 16 seq chunks of 128
NKC = S // P            # 16 key chunks of 128

L2E = float(np.log2(np.e))
LN2 = float(np.log(2.0))
C_SCH = 16250.5
# Every kc runs one exp tile on ACT and one on the DVE (Schraudolph), so the
# two engines stream in parallel.  kc in DVE_KCS -> head-odd tile on DVE;
# else head-even on DVE.  Alternating keeps the approximation error split
# evenly across both heads.  f_schraudolph = 0.5 structurally.
DVE_KCS = frozenset(range(0, NKC, 2))

_cache = {}


def _build():
    import concourse.bass as bass
    import concourse.bacc as bacc
    import concourse.mybir as mybir
    import concourse.tile as tile
    from contextlib import ExitStack

    f32 = mybir.dt.float32
    bf16 = mybir.dt.bfloat16
    i16 = mybir.dt.int16
    AF = mybir.ActivationFunctionType
    MUL = mybir.AluOpType.mult
    ADD = mybir.AluOpType.add

    nc = bacc.Bacc("TRN2", target_bir_lowering=False, debug=False,
                   num_devices=NCORES)

    # Activations/weights arrive pre-permuted from the host so every DMA is
    # a dense per-partition burst (8KB lines) instead of 1KB strided lines.
    qT = nc.dram_tensor("qT", [NSC, P, NDM, 512], bf16, kind="ExternalInput").ap()
    kT = nc.dram_tensor("kT", [NSC, P, NDM, 512], bf16, kind="ExternalInput").ap()
    vT = nc.dram_tensor("vT", [NSC, P, NDM, 512], bf16, kind="ExternalInput").ap()
    wq = nc.dram_tensor("wq", [P, NDM, PROJ], bf16, kind="ExternalInput").ap()
    wk = nc.dram_tensor("wk", [P, NDM, PROJ], bf16, kind="ExternalInput").ap()
    wv = nc.dram_tensor("wv", [P, NDM, PROJ], bf16, kind="ExternalInput").ap()
    wo = nc.dram_tensor("wo", [P, NPC, D], bf16, kind="ExternalInput").ap()
    bq2 = nc.dram_tensor("bq2", [P, NPC], f32, kind="ExternalInput").ap()
    bk2 = nc.dram_tensor("bk2", [P, NPC], f32, kind="ExternalInput").ap()
    mk = nc.dram_tensor("mk", [P, NSO], f32, kind="ExternalInput").ap()
    sel = nc.dram_tensor("sel", [P, 128], bf16, kind="ExternalInput").ap()
    out = nc.dram_tensor("out", [S, D], f32, kind="ExternalOutput").ap()

    with tile.TileContext(nc) as tc, ExitStack() as ctx:
        cpool = ctx.enter_context(tc.tile_pool(name="const", bufs=1))
        sel_sb = cpool.tile([P, 128], bf16)
        nc.sync.dma_start(sel_sb[:], sel)
        bq_sb = cpool.tile([P, NPC], f32)
        nc.sync.dma_start(bq_sb[:], bq2)
        bk_sb = cpool.tile([P, NPC], f32)
        nc.sync.dma_start(bk_sb[:], bk2)
        mk_sb = cpool.tile([P, NSO], f32)
        nc.sync.dma_start(mk_sb[:], mk)
        # l values land in rows {0, 32}; other rows must stay finite for the
        # sel broadcast matmul.
        Lsb = cpool.tile([P, 1024], bf16)
        nc.gpsimd.memset(Lsb[:], 0.0)
        ones8 = cpool.tile([P, HPC], bf16)
        nc.gpsimd.memset(ones8[:], 1.0)

        respool = ctx.enter_context(tc.tile_pool(name="res", bufs=1))
        # Q^T pair-stacked: rows 0-63 = head 2*pr dims, 64-127 = head
        # 2*pr+1.  K^T stored per head on the full 128-partition contraction
        # range (even heads rows 0-63, odd heads 64-127, rest zero) so the
        # score matmuls are full-array standard-mode matmuls: partial-array
        # tiling modes do not register as PE-busy in the HAM activity
        # monitor and leave the clock gate throttled at 1.2 GHz (measured:
        # 414us throttle-active with 64-row tiling vs 18us without).
        QT_sb = respool.tile([P, NPC, S], bf16)
        KT_sb = respool.tile([P, HPC, S], bf16)
        nc.gpsimd.memset(KT_sb[:], 0.0)
        # V with an interleaved mask column per head: head h occupies cols
        # [h*65, h*65+64) and col h*65+64 == mask (the masked softmax
        # denominator rides the AV matmul as output partition 64).
        V_sb = respool.tile([P, NSO, HPC * (DK + 1)], bf16)
        for so in range(NSO):
            # mask columns filled on gpsimd so the DVE queue stays clear for
            # the K-projection bias adds.
            nc.gpsimd.tensor_scalar_mul(
                V_sb[:, so, :].rearrange("p (h w) -> p h w", w=DK + 1)[:, :, DK],
                ones8[:], mk_sb[:, so:so + 1])
        AT_sb = respool.tile([P, NPC, S], bf16)   # normalized A^T

        wopool = ctx.enter_context(tc.tile_pool(name="wo", bufs=1))
        wo_sb = wopool.tile([P, NPC, D], bf16)

        npool = ctx.enter_context(tc.tile_pool(name="norm", bufs=4))
        rcpool = ctx.enter_context(tc.tile_pool(name="rc", bufs=2))
        epA = ctx.enter_context(tc.tile_pool(name="expA", bufs=5))
        epB = ctx.enter_context(tc.tile_pool(name="expB", bufs=6))
        opool = ctx.enter_context(tc.tile_pool(name="ostage", bufs=4))

        # Weight + activation staging pools live for the whole kernel: wq is
        # needed for trailing Q projections inside phase B.
        wpool = ctx.enter_context(tc.tile_pool(name="w", bufs=2))
        apool = ctx.enter_context(tc.tile_pool(name="act", bufs=4))

        # ---------------- Phase A: K, V, Q(pair0) ----------------
        with ExitStack() as ctxA:
            psA = ctxA.enter_context(
                tc.tile_pool(name="psA", bufs=4, space="PSUM"))

            # K projection -> pair-packed KT_sb
            wk_sb = wpool.tile([P, NDM, PROJ], bf16, tag="w", name="wk_sb")
            nc.sync.dma_start(wk_sb[:], wk)
            for sc in range(NSC):
                a_sb = apool.tile([P, NDM, 512], bf16, tag="a", name="a_sb")
                nc.sync.dma_start(a_sb[:], kT[sc])
                for pc in range(NPC):
                    ps = psA.tile([P, 512], f32, tag="pp", name="psa")
                    for dc in range(NDM):
                        nc.tensor.matmul(
                            ps,
                            lhsT=wk_sb[:, dc, pc * P:(pc + 1) * P],
                            rhs=a_sb[:, dc, :],
                            start=(dc == 0), stop=(dc == NDM - 1))
                    for half in range(2):
                        lo = half * 64
                        nc.vector.tensor_scalar_add(
                            KT_sb[lo:lo + 64, 2 * pc + half,
                                  sc * 512:(sc + 1) * 512],
                            ps[lo:lo + 64, :],
                            bk_sb[lo:lo + 64, pc:pc + 1])

            # V projection (mask folded in)
            wv_sb = wpool.tile([P, NDM, PROJ], bf16, tag="w", name="wv_sb")
            nc.sync.dma_start(wv_sb[:], wv)
            for sc in range(NSC):
                a_sb = apool.tile([P, NDM, 512], bf16, tag="a", name="a_sb")
                nc.sync.dma_start(a_sb[:], vT[sc])
                for so4 in range(4):
                    so = sc * 4 + so4
                    ps = psA.tile([P, 512], f32, tag="pp", name="psa")
                    for dc in range(NDM):
                        nc.tensor.matmul(
                            ps,
                            lhsT=a_sb[:, dc, so4 * P:(so4 + 1) * P],
                            rhs=wv_sb[:, dc, :],
                            start=(dc == 0), stop=(dc == NDM - 1))
                    nc.vector.tensor_scalar_mul(
                        V_sb[:, so, :].rearrange(
                            "p (h w) -> p h w", w=DK + 1)[:, :, 0:DK],
                        ps.rearrange("p (h w) -> p h w", w=DK),
                        mk_sb[:, so:so + 1])

            # Q projection, pair 0 only; wq stays resident for pairs 1-3.
            wq_sb = wpool.tile([P, NDM, PROJ], bf16, tag="w", name="wq_sb")
            nc.sync.dma_start(wq_sb[:], wq)
            for sc in range(NSC):
                a_sb = apool.tile([P, NDM, 512], bf16, tag="a", name="a_sb")
                nc.sync.dma_start(a_sb[:], qT[sc])
                ps = psA.tile([P, 512], f32, tag="pp", name="psa")
                for dc in range(NDM):
                    nc.tensor.matmul(
                        ps,
                        lhsT=wq_sb[:, dc, 0:P],
                        rhs=a_sb[:, dc, :],
                        start=(dc == 0), stop=(dc == NDM - 1))
                nc.vector.tensor_scalar_add(
                    QT_sb[:, 0, sc * 512:(sc + 1) * 512], ps,
                    bq_sb[:, 0:1])
            nc.sync.dma_start(wo_sb[:], wo)

        # ---------------- Phase B ----------------
        psSA = ctx.enter_context(tc.tile_pool(name="psSA", bufs=1, space="PSUM"))
        psSB = ctx.enter_context(tc.tile_pool(name="psSB", bufs=1, space="PSUM"))
        psAcA = ctx.enter_context(tc.tile_pool(name="psAcA", bufs=1, space="PSUM"))
        psAcB = ctx.enter_context(tc.tile_pool(name="psAcB", bufs=1, space="PSUM"))

        qstage = {}

        def qprefetch(pc):
            """DMA the qT chunks for pair pc's trailing projection."""
            tiles = []
            for sc in range(NSC):
                a_sb = apool.tile([P, NDM, 512], bf16, tag="a", name="a_q")
                nc.sync.dma_start(a_sb[:], qT[sc])
                tiles.append(a_sb)
            qstage[pc] = tiles

        def emit_qproj(pc):
            """Trailing Q projection for pair pc (borrows psSA/psSB)."""
            tiles = qstage.pop(pc)
            for sc in range(NSC):
                a_sb = tiles[sc]
                pool = psSB if sc % 2 == 0 else psSA
                ps = pool.tile([P, 512], f32, tag="s", name="ps_q")
                for dc in range(NDM):
                    nc.tensor.matmul(
                        ps,
                        lhsT=wq_sb[:, dc, pc * P:(pc + 1) * P],
                        rhs=a_sb[:, dc, :],
                        start=(dc == 0), stop=(dc == NDM - 1))
                nc.vector.tensor_scalar_add(
                    QT_sb[:, pc, sc * 512:(sc + 1) * 512], ps,
                    bq_sb[:, pc:pc + 1])

        def emit_c_chunk(so, oc, pool):
            """Output projection chunk [128q, 512o] (borrows a score pool)."""
            ps = pool.tile([P, 512], f32, tag="s", name="ps_c")
            for pc in range(NPC):
                nc.tensor.matmul(
                    ps,
                    lhsT=AT_sb[:, pc, so * P:(so + 1) * P],
                    rhs=wo_sb[:, pc, oc * 512:(oc + 1) * 512],
                    start=(pc == 0), stop=(pc == NPC - 1))
            ost = opool.tile([P, 512], f32, tag="o", name="ost")
            nc.scalar.copy(ost, ps)
            nc.sync.dma_start(
                out[so * P:(so + 1) * P, oc * 512:(oc + 1) * 512], ost)

        def emit_scores_exp(pr, qc, kc):
            """Standard-mode pair scores + the pair's exp, one tile/engine.

            The DVE tile is processed in two N=512 halves so its score bank
            frees early enough for the next kc's matmuls.
            """
            sa = psSA.tile([P, 1024], f32, tag="s", name="sa")
            sb = psSB.tile([P, 1024], f32, tag="s", name="sb")
            ea = epA.tile([P, 1024], bf16, tag="e", name="ea")
            eb = epB.tile([P, 1024], bf16, tag="e", name="eb")
            if kc in DVE_KCS:
                act_t, act_ps, dve_t, dve_ps = ea, sa, eb, sb
            else:
                act_t, act_ps, dve_t, dve_ps = eb, sb, ea, sa
            for sub in range(2):
                ssl = slice(sub * 512, (sub + 1) * 512)
                cols = slice(qc * 1024 + sub * 512, qc * 1024 + (sub + 1) * 512)
                nc.tensor.matmul(
                    sa[:, ssl],
                    lhsT=KT_sb[:, 2 * pr, kc * P:(kc + 1) * P],
                    rhs=QT_sb[:, pr, cols],
                    start=True, stop=True)
                nc.tensor.matmul(
                    sb[:, ssl],
                    lhsT=KT_sb[:, 2 * pr + 1, kc * P:(kc + 1) * P],
                    rhs=QT_sb[:, pr, cols],
                    start=True, stop=True)
                nc.vector.tensor_scalar(
                    dve_t[:, ssl].bitcast(i16), dve_ps[:, ssl],
                    128.0, C_SCH, MUL, ADD)
                if sub == 1:
                    nc.scalar.activation(act_t, act_ps, AF.Exp, scale=LN2)
            return ea, eb

        def emit_av(acc, h, e, kc, start, stop):
            for sub in range(2):
                nc.tensor.matmul(
                    acc[0:DK + 1, sub * 512:(sub + 1) * 512],
                    lhsT=V_sb[:, kc, h * (DK + 1):(h + 1) * (DK + 1)],
                    rhs=e[:, sub * 512:(sub + 1) * 512],
                    start=start, stop=stop)

        def block_tails(pr, qc, accA, accB, esA, esB):
            """Last AVs, PSUM evacuation, l rows, 1/l, normalize -> AT_sb.

            Evacuation copies run on ACT (which has slack), the all-SBUF
            normalize multiplies on GPSIMD (idle otherwise), keeping the DVE
            free for the exp stream.  bc reuses accB's banks so both score
            banks are immediately available for boundary chunks.
            """
            emit_av(accA, 2 * pr + 0, esA[NKC - 2], NKC - 2, False, False)
            emit_av(accA, 2 * pr + 0, esA[NKC - 1], NKC - 1, False, True)
            emit_av(accB, 2 * pr + 1, esB[NKC - 2], NKC - 2, False, False)
            emit_av(accB, 2 * pr + 1, esB[NKC - 1], NKC - 1, False, True)
            nc.scalar.copy(Lsb[0:1, :], accA[DK:DK + 1, :])
            nc.scalar.copy(Lsb[32:33, :], accB[DK:DK + 1, :])
            atA = npool.tile([P, 1024], f32, tag="at", name="atA")
            nc.scalar.copy(atA[0:64, :], accA[0:64, :])
            atB = npool.tile([P, 1024], f32, tag="at", name="atB")
            nc.scalar.copy(atB[64:128, :], accB[0:64, :])
            # broadcast l to the heads' dim rows, reciprocal, normalize
            bc = psAcB.tile([P, 1024], f32, tag="av", name="bc")
            for sub in range(2):
                nc.tensor.matmul(
                    bc[:, sub * 512:(sub + 1) * 512],
                    lhsT=sel_sb[:],
                    rhs=Lsb[:, sub * 512:(sub + 1) * 512],
                    start=True, stop=True)
            rc = rcpool.tile([P, 1024], f32, tag="rc", name="rc")
            nc.vector.reciprocal_approx_fast(out=rc[:], in_=bc[:])
            nc.gpsimd.tensor_mul(
                AT_sb[0:64, pr, qc * 1024:(qc + 1) * 1024],
                atA[0:64, :], rc[0:64, :])
            nc.gpsimd.tensor_mul(
                AT_sb[64:128, pr, qc * 1024:(qc + 1) * 1024],
                atB[64:128, :], rc[64:128, :])

        blocks = [(qc, pr) for qc in range(2) for pr in range(NPC)]
        for bi, (qc, pr) in enumerate(blocks):
            esA = [None] * NKC
            esB = [None] * NKC
            accA = accB = None
            for kc in range(NKC):
                esA[kc], esB[kc] = emit_scores_exp(pr, qc, kc)
                if kc == 6 and qc == 0 and pr < 3:
                    qprefetch(pr + 1)
                if kc == 2:
                    accA = psAcA.tile([P, 1024], f32, tag="av", name="accA")
                    accB = psAcB.tile([P, 1024], f32, tag="av", name="accB")
                    emit_av(accA, 2 * pr + 0, esA[0], 0, True, False)
                    emit_av(accB, 2 * pr + 1, esB[0], 0, True, False)
                elif kc > 2:
                    emit_av(accA, 2 * pr + 0, esA[kc - 2], kc - 2, False, False)
                    emit_av(accB, 2 * pr + 1, esB[kc - 2], kc - 2, False, False)
            block_tails(pr, qc, accA, accB, esA, esB)

            # boundary work: trailing Q projections during qc0, qc0's output
            # projection during qc1 boundaries; qc1's C lands in the drain.
            tc.no_sync_barrier()
            if qc == 0 and pr < 3:
                emit_qproj(pr + 1)
            if bi >= 3:
                # 4 qc0 C-chunks at each of the last 5 boundaries handles
                # 16 chunks by the end of block (1, pr2); emit 4 per boundary
                # starting after block (0, pr3).
                cidx = (bi - 3) * 4
                if cidx < 16:
                    for j in range(cidx, cidx + 4):
                        so, oc = j // 2, j % 2
                        emit_c_chunk(so, oc, psSA if j % 2 == 0 else psSB)

        # drain: qc1's output projection
        tc.no_sync_barrier()
        for j in range(16):
            so, oc = 8 + j // 2, j % 2
            emit_c_chunk(so, oc, psSA if j % 2 == 0 else psSB)

    nc.compile()
    return nc


def _get_nc():
    if "nc" not in _cache:
        _cache["nc"] = _build()
    return _cache["nc"]


def make_in_maps(q, k, v, mask, Wq, bq, Wk, bk, Wv, bv, Wo, bo):
    """Host-side sharding: slice/transpose the full inputs per core."""
    import ml_dtypes
    f = np.float32
    bf = ml_dtypes.bfloat16
    q = np.asarray(q, dtype=f)
    k = np.asarray(k, dtype=f)
    v = np.asarray(v, dtype=f)
    Wq = np.asarray(Wq, dtype=f) * (L2E / 8.0)   # exp2-domain prescale
    Wk = np.asarray(Wk, dtype=f)
    Wv = np.asarray(Wv, dtype=f)
    Wo = np.asarray(Wo, dtype=f)
    bq = np.asarray(bq, dtype=f) * (L2E / 8.0)
    bk = np.asarray(bk, dtype=f)
    mask = np.asarray(mask)

    sel = np.zeros((P, 128), dtype=f)
    sel[0, 0:64] = 1.0
    sel[32, 64:128] = 1.0

    def act_perm(x):
        # [S, D] -> [NSC, P, NDM, 512]: chunk sc holds x^T[d, sc*512 + j]
        # with d = o*128 + p, laid out as one dense 8KB line per partition.
        return np.ascontiguousarray(
            x.reshape(NSC, 512, NDM, P).transpose(0, 3, 2, 1)).astype(bf)

    def w_perm(w):
        # [D, PROJ] -> [P, NDM, PROJ]
        return np.ascontiguousarray(
            w.reshape(NDM, P, -1).transpose(1, 0, 2)).astype(bf)

    in_maps = []
    for c in range(NCORES):
        b, hg = divmod(c, 2)
        cols = slice(hg * PROJ, (hg + 1) * PROJ)
        mvals = (mask[b, 0, 0, :] != 0).astype(f)            # [S]
        mk2 = np.ascontiguousarray(mvals.reshape(NSO, P).T)  # [P, NSO]
        in_maps.append({
            "qT": act_perm(q[b]),
            "kT": act_perm(k[b]),
            "vT": act_perm(v[b]),
            "wq": w_perm(Wq[:, cols]),
            "wk": w_perm(Wk[:, cols]),
            "wv": w_perm(Wv[:, cols]),
            "wo": np.ascontiguousarray(
                Wo[cols, :].reshape(NPC, P, D).transpose(1, 0, 2)).astype(bf),
            "bq2": np.ascontiguousarray(bq[cols].reshape(NPC, P).T),
            "bk2": np.ascontiguousarray(bk[cols].reshape(NPC, P).T),
            "mk": mk2,
            "sel": sel.astype(bf),
        })
    return in_maps


def combine_outputs(parts, Wv_bv_Wo_bo):
    """Sum the two head-group partials per batch, add bv @ Wo + bo."""
    bv, Wo, bo = Wv_bv_Wo_bo
    bo_eff = (np.asarray(bv, np.float32) @ np.asarray(Wo, np.float32)
              + np.asarray(bo, np.float32))
    out = np.empty((B, S, D), dtype=np.float32)
    for b in range(B):
        out[b] = parts[2 * b] + parts[2 * b + 1] + bo_eff
    return out


def _install_axon_ntff_hook():
    """The agent image's antenv lacks axon_hooks; synthesize it and register
    the ctypes NTFF profile hook from trn_boot so trace=True works."""
    import sys
    import types
    if "antenv.axon_hooks" in sys.modules:
        return
    try:
        from trn_agent_boot.trn_boot import _ntff_profile_via_ctypes
        hook = _ntff_profile_via_ctypes("/opt/axon/libaxon_pjrt.so")
    except Exception:
        hook = None
    mod = types.ModuleType("antenv.axon_hooks")
    mod._hook = hook
    mod.get_axon_ntff_profile_hook = lambda: mod._hook
    mod.set_axon_ntff_profile_hook = lambda h: setattr(mod, "_hook", h)
    sys.modules["antenv.axon_hooks"] = mod
    import concourse.bass_utils as bu
    bu.upload_artifacts = lambda tmpdir: str(tmpdir)


def kernel(q, k, v, mask, Wq, bq, Wk, bk, Wv, bv, Wo, bo):
    from concourse.bass_utils import run_bass_kernel_spmd

    nc = _get_nc()
    in_maps = make_in_maps(q, k, v, mask, Wq, bq, Wk, bk, Wv, bv, Wo, bo)
    trace = bool(int(os.environ.get("KERNEL_TRACE", "0")))
    if trace:
        try:
            _install_axon_ntff_hook()
        except Exception:
            trace = False
    try:
        res = run_bass_kernel_spmd(
            nc, in_maps, list(range(NCORES)), trace=trace,
            tmpdir=os.environ.get("KERNEL_TRACE_DIR") or None)
    except Exception:
        if not trace:
            raise
        res = run_bass_kernel_spmd(nc, in_maps, list(range(NCORES)), trace=False)
    _cache["last_result"] = res
    parts = [res.results[c]["out"] for c in range(NCORES)]
    return combine_outputs(parts, (bv, Wo, bo))


# revision 21
# speedup vs baseline: 1.5651x; 1.2241x over previous
"""Multi-head attention (B=4, S=2048, D=1024, H=16) on 8 Trainium2 cores.

Sharding: core c handles batch b = c//2 and head-group hg = c%2 (8 of the 16
heads, 512 of the 1024 projection dims).  Host sums the two head-group
partials per batch (the "all-reduce after w_o") and adds bv@Wo + bo.

v2 design (vs the 425us baseline):
  * Pair-blocks: per (qc, pr) block both heads' scores are computed with
    64-row PE tiling (K=64 row tiles run concurrently -> 2x score matmul
    throughput; probe-measured 110ns/MM vs 216 standard).
  * Q is pre-scaled by log2(e)/8 on the host (folded into Wq/bq) so scores
    are in the exp2 domain.  exp for head-even tiles runs on ACT
    (exp(y*ln2)); head-odd tiles mostly run on the DVE as a one-instruction
    Schraudolph exp2 (tensor_scalar mult+add -> int16 bits == bf16 exp2
    approximation, ~3.3% max elementwise, cancels through softmax to ~1e-2
    final).  This splits the 293us exp load across two engines.
  * PSUM: SA + SB score tiles + two AV accumulators = exactly 8 banks.
    The l (softmax denominator) rides the AV matmul as V's 65th column.
  * Phase A emits K, V, Q(pair0) only; Q(pair 1-3) projections trail at
    qc0 block boundaries, C chunks of qc0 at qc1 boundaries, C of qc1 in
    the drain.  1/l uses reciprocal_approx_fast (5x the DVE reciprocal).

All matmuls are bf16 with fp32 PSUM accumulation.
"""

import os
import numpy as np

B, S, D = 4, 2048, 1024
H, DK = 16, 64
P = 128
NCORES = 8
HPC = H // 2            # heads per core
PROJ = HPC * DK         # 512 projection dims per core
NDM = D // P            # 8 d_model chunks
NPC = PROJ // P         # 4 head-pair chunks
NSC = S // 512          # 4 seq chunks of 512
NSO = S // P            # 16 seq chunks of 128
NKC = S // P            # 16 key chunks of 128

L2E = float(np.log2(np.e))
LN2 = float(np.log(2.0))
C_SCH = 16250.5
# Every kc runs one exp tile on ACT and one on the DVE (Schraudolph), so the
# two engines stream in parallel.  kc in DVE_KCS -> head-odd tile on DVE;
# else head-even on DVE.  Alternating keeps the approximation error split
# evenly across both heads.  f_schraudolph = 0.5 structurally.
DVE_KCS = frozenset(range(0, NKC, 2))

_cache = {}


def _build():
    import concourse.bass as bass
    import concourse.bacc as bacc
    import concourse.mybir as mybir
    import concourse.tile as tile
    from contextlib import ExitStack

    f32 = mybir.dt.float32
    bf16 = mybir.dt.bfloat16
    i16 = mybir.dt.int16
    AF = mybir.ActivationFunctionType
    MUL = mybir.AluOpType.mult
    ADD = mybir.AluOpType.add

    nc = bacc.Bacc("TRN2", target_bir_lowering=False, debug=False,
                   num_devices=NCORES)

    # Activations/weights arrive pre-permuted from the host so every DMA is
    # a dense per-partition burst (8KB lines) instead of 1KB strided lines.
    qT = nc.dram_tensor("qT", [NSC, P, NDM, 512], bf16, kind="ExternalInput").ap()
    kT = nc.dram_tensor("kT", [NSC, P, NDM, 512], bf16, kind="ExternalInput").ap()
    vT = nc.dram_tensor("vT", [NSC, P, NDM, 512], bf16, kind="ExternalInput").ap()
    wq = nc.dram_tensor("wq", [P, NDM, PROJ], bf16, kind="ExternalInput").ap()
    wk = nc.dram_tensor("wk", [P, NDM, PROJ], bf16, kind="ExternalInput").ap()
    wv = nc.dram_tensor("wv", [P, NDM, PROJ], bf16, kind="ExternalInput").ap()
    wo = nc.dram_tensor("wo", [P, NPC, D], bf16, kind="ExternalInput").ap()
    bq2 = nc.dram_tensor("bq2", [P, NPC], f32, kind="ExternalInput").ap()
    bk2 = nc.dram_tensor("bk2", [P, NPC], f32, kind="ExternalInput").ap()
    mk = nc.dram_tensor("mk", [P, NSO], f32, kind="ExternalInput").ap()
    sel = nc.dram_tensor("sel", [P, 128], bf16, kind="ExternalInput").ap()
    out = nc.dram_tensor("out", [S, D], f32, kind="ExternalOutput").ap()

    with tile.TileContext(nc) as tc, ExitStack() as ctx:
        cpool = ctx.enter_context(tc.tile_pool(name="const", bufs=1))
        sel_sb = cpool.tile([P, 128], bf16)
        nc.sync.dma_start(sel_sb[:], sel)
        bq_sb = cpool.tile([P, NPC], f32)
        nc.sync.dma_start(bq_sb[:], bq2)
        bk_sb = cpool.tile([P, NPC], f32)
        nc.sync.dma_start(bk_sb[:], bk2)
        mk_sb = cpool.tile([P, NSO], f32)
        nc.sync.dma_start(mk_sb[:], mk)
        # l values land in rows {0, 32}; other rows must stay finite for the
        # sel broadcast matmul.
        Lsb = cpool.tile([P, 1024], bf16)
        nc.gpsimd.memset(Lsb[:], 0.0)
        ones8 = cpool.tile([P, HPC], bf16)
        nc.gpsimd.memset(ones8[:], 1.0)

        respool = ctx.enter_context(tc.tile_pool(name="res", bufs=1))
        # Q^T pair-stacked: rows 0-63 = head 2*pr dims, 64-127 = head
        # 2*pr+1.  K^T stored per head on the full 128-partition contraction
        # range (even heads rows 0-63, odd heads 64-127, rest zero) so the
        # score matmuls are full-array standard-mode matmuls: partial-array
        # tiling modes do not register as PE-busy in the HAM activity
        # monitor and leave the clock gate throttled at 1.2 GHz (measured:
        # 414us throttle-active with 64-row tiling vs 18us without).
        QT_sb = respool.tile([P, NPC, S], bf16)
        KT_sb = respool.tile([P, HPC, S], bf16)
        nc.vector.memset(KT_sb[:], 0.0)
        # V with an interleaved mask column per head: head h occupies cols
        # [h*65, h*65+64) and col h*65+64 == mask (the masked softmax
        # denominator rides the AV matmul as output partition 64).
        V_sb = respool.tile([P, NSO, HPC * (DK + 1)], bf16)
        for so in range(NSO):
            # mask columns filled on gpsimd so the DVE queue stays clear for
            # the K-projection bias adds.
            nc.gpsimd.tensor_scalar_mul(
                V_sb[:, so, :].rearrange("p (h w) -> p h w", w=DK + 1)[:, :, DK],
                ones8[:], mk_sb[:, so:so + 1])
        AT_sb = respool.tile([P, NPC, S], bf16)   # normalized A^T

        wopool = ctx.enter_context(tc.tile_pool(name="wo", bufs=1))
        wo_sb = wopool.tile([P, NPC, D], bf16)

        npool = ctx.enter_context(tc.tile_pool(name="norm", bufs=4))
        rcpool = ctx.enter_context(tc.tile_pool(name="rc", bufs=2))
        epA = ctx.enter_context(tc.tile_pool(name="expA", bufs=5))
        epB = ctx.enter_context(tc.tile_pool(name="expB", bufs=6))
        opool = ctx.enter_context(tc.tile_pool(name="ostage", bufs=4))

        # Weight + activation staging pools live for the whole kernel: wq is
        # needed for trailing Q projections inside phase B.
        wpool = ctx.enter_context(tc.tile_pool(name="w", bufs=2))
        apool = ctx.enter_context(tc.tile_pool(name="act", bufs=4))

        # ---------------- Phase A: K, V, Q(pair0) ----------------
        with ExitStack() as ctxA:
            psA = ctxA.enter_context(
                tc.tile_pool(name="psA", bufs=4, space="PSUM"))

            # K projection -> pair-packed KT_sb
            wk_sb = wpool.tile([P, NDM, PROJ], bf16, tag="w", name="wk_sb")
            nc.sync.dma_start(wk_sb[:], wk)
            for sc in range(NSC):
                a_sb = apool.tile([P, NDM, 512], bf16, tag="a", name="a_sb")
                nc.sync.dma_start(a_sb[:], kT[sc])
                for pc in range(NPC):
                    ps = psA.tile([P, 512], f32, tag="pp", name="psa")
                    for dc in range(NDM):
                        nc.tensor.matmul(
                            ps,
                            lhsT=wk_sb[:, dc, pc * P:(pc + 1) * P],
                            rhs=a_sb[:, dc, :],
                            start=(dc == 0), stop=(dc == NDM - 1))
                    for half in range(2):
                        lo = half * 64
                        nc.vector.tensor_scalar_add(
                            KT_sb[lo:lo + 64, 2 * pc + half,
                                  sc * 512:(sc + 1) * 512],
                            ps[lo:lo + 64, :],
                            bk_sb[lo:lo + 64, pc:pc + 1])

            # V projection (mask folded in)
            wv_sb = wpool.tile([P, NDM, PROJ], bf16, tag="w", name="wv_sb")
            nc.sync.dma_start(wv_sb[:], wv)
            for sc in range(NSC):
                a_sb = apool.tile([P, NDM, 512], bf16, tag="a", name="a_sb")
                nc.sync.dma_start(a_sb[:], vT[sc])
                for so4 in range(4):
                    so = sc * 4 + so4
                    ps = psA.tile([P, 512], f32, tag="pp", name="psa")
                    for dc in range(NDM):
                        nc.tensor.matmul(
                            ps,
                            lhsT=a_sb[:, dc, so4 * P:(so4 + 1) * P],
                            rhs=wv_sb[:, dc, :],
                            start=(dc == 0), stop=(dc == NDM - 1))
                    nc.vector.tensor_scalar_mul(
                        V_sb[:, so, :].rearrange(
                            "p (h w) -> p h w", w=DK + 1)[:, :, 0:DK],
                        ps.rearrange("p (h w) -> p h w", w=DK),
                        mk_sb[:, so:so + 1])

            # Q projection, pair 0 only; wq stays resident for pairs 1-3.
            wq_sb = wpool.tile([P, NDM, PROJ], bf16, tag="w", name="wq_sb")
            nc.sync.dma_start(wq_sb[:], wq)
            for sc in range(NSC):
                a_sb = apool.tile([P, NDM, 512], bf16, tag="a", name="a_sb")
                nc.sync.dma_start(a_sb[:], qT[sc])
                ps = psA.tile([P, 512], f32, tag="pp", name="psa")
                for dc in range(NDM):
                    nc.tensor.matmul(
                        ps,
                        lhsT=wq_sb[:, dc, 0:P],
                        rhs=a_sb[:, dc, :],
                        start=(dc == 0), stop=(dc == NDM - 1))
                nc.vector.tensor_scalar_add(
                    QT_sb[:, 0, sc * 512:(sc + 1) * 512], ps,
                    bq_sb[:, 0:1])
            nc.sync.dma_start(wo_sb[:], wo)

        # ---------------- Phase B ----------------
        # Four single-bank score pools: each 512-column half of each head's
        # score tile is its own PSUM tile, so the next kc's matmul into a
        # half only waits for the one exp instruction that read that half.
        psS4 = [ctx.enter_context(
            tc.tile_pool(name=f"psS{i}", bufs=1, space="PSUM"))
            for i in range(4)]
        psAcA = ctx.enter_context(tc.tile_pool(name="psAcA", bufs=1, space="PSUM"))
        psAcB = ctx.enter_context(tc.tile_pool(name="psAcB", bufs=1, space="PSUM"))

        qstage = {}

        def qprefetch(pc):
            """DMA the qT chunks for pair pc's trailing projection."""
            tiles = []
            for sc in range(NSC):
                a_sb = apool.tile([P, NDM, 512], bf16, tag="a", name="a_q")
                nc.sync.dma_start(a_sb[:], qT[sc])
                tiles.append(a_sb)
            qstage[pc] = tiles

        def emit_qproj(pc):
            """Trailing Q projection for pair pc (borrows psSA/psSB)."""
            tiles = qstage.pop(pc)
            for sc in range(NSC):
                a_sb = tiles[sc]
                pool = psS4[sc]
                ps = pool.tile([P, 512], f32, tag="s", name="ps_q")
                for dc in range(NDM):
                    nc.tensor.matmul(
                        ps,
                        lhsT=wq_sb[:, dc, pc * P:(pc + 1) * P],
                        rhs=a_sb[:, dc, :],
                        start=(dc == 0), stop=(dc == NDM - 1))
                nc.vector.tensor_scalar_add(
                    QT_sb[:, pc, sc * 512:(sc + 1) * 512], ps,
                    bq_sb[:, pc:pc + 1])

        def emit_c_chunk(so, oc, pool):
            """Output projection chunk [128q, 512o] (borrows a score pool)."""
            ps = pool.tile([P, 512], f32, tag="s", name="ps_c")
            for pc in range(NPC):
                nc.tensor.matmul(
                    ps,
                    lhsT=AT_sb[:, pc, so * P:(so + 1) * P],
                    rhs=wo_sb[:, pc, oc * 512:(oc + 1) * 512],
                    start=(pc == 0), stop=(pc == NPC - 1))
            ost = opool.tile([P, 512], f32, tag="o", name="ost")
            nc.scalar.copy(ost, ps)
            nc.sync.dma_start(
                out[so * P:(so + 1) * P, oc * 512:(oc + 1) * 512], ost)

        def emit_scores_exp(pr, qc, kc):
            """Standard-mode pair scores + the pair's exp, one tile/engine.

            Both engines consume their score halves as separate N=512
            instructions against separate single-bank PSUM tiles; the
            ACT-side tile is computed first since its chain is longest.
            """
            ea = epA.tile([P, 1024], bf16, tag="e", name="ea")
            eb = epB.tile([P, 1024], bf16, tag="e", name="eb")
            if kc in DVE_KCS:
                order = ((0, ea, True), (1, eb, False))
            else:
                order = ((1, eb, True), (0, ea, False))
            for hi, et, on_act in order:
                for sub in range(2):
                    ssl = slice(sub * 512, (sub + 1) * 512)
                    cols = slice(qc * 1024 + sub * 512,
                                 qc * 1024 + (sub + 1) * 512)
                    ps = psS4[2 * hi + sub].tile([P, 512], f32, tag="s",
                                                 name="ssc")
                    nc.tensor.matmul(
                        ps,
                        lhsT=KT_sb[:, 2 * pr + hi, kc * P:(kc + 1) * P],
                        rhs=QT_sb[:, pr, cols],
                        start=True, stop=True)
                    if on_act:
                        nc.scalar.activation(et[:, ssl], ps, AF.Exp,
                                             scale=LN2)
                    else:
                        nc.vector.tensor_scalar(
                            et[:, ssl].bitcast(i16), ps,
                            128.0, C_SCH, MUL, ADD)
            return ea, eb

        def emit_av(acc, h, e, kc, start, stop):
            for sub in range(2):
                nc.tensor.matmul(
                    acc[0:DK + 1, sub * 512:(sub + 1) * 512],
                    lhsT=V_sb[:, kc, h * (DK + 1):(h + 1) * (DK + 1)],
                    rhs=e[:, sub * 512:(sub + 1) * 512],
                    start=start, stop=stop)

        def block_tails(pr, qc, accA, accB, esA, esB):
            """Last AVs, PSUM evacuation, l rows, 1/l, normalize -> AT_sb.

            Evacuation copies run on ACT (which has slack), the all-SBUF
            normalize multiplies on GPSIMD (idle otherwise), keeping the DVE
            free for the exp stream.  bc reuses accB's banks so both score
            banks are immediately available for boundary chunks.
            """
            emit_av(accA, 2 * pr + 0, esA[NKC - 2], NKC - 2, False, False)
            emit_av(accA, 2 * pr + 0, esA[NKC - 1], NKC - 1, False, True)
            emit_av(accB, 2 * pr + 1, esB[NKC - 2], NKC - 2, False, False)
            emit_av(accB, 2 * pr + 1, esB[NKC - 1], NKC - 1, False, True)
            nc.vector.tensor_copy(Lsb[0:1, :], accA[DK:DK + 1, :])
            nc.vector.tensor_copy(Lsb[32:33, :], accB[DK:DK + 1, :])
            atA = npool.tile([P, 1024], f32, tag="at", name="atA")
            nc.scalar.copy(atA[0:64, :], accA[0:64, :])
            atB = npool.tile([P, 1024], f32, tag="at", name="atB")
            nc.scalar.copy(atB[64:128, :], accB[0:64, :])
            # broadcast l to the heads' dim rows, reciprocal, normalize
            bc = psAcB.tile([P, 1024], f32, tag="av", name="bc")
            for sub in range(2):
                nc.tensor.matmul(
                    bc[:, sub * 512:(sub + 1) * 512],
                    lhsT=sel_sb[:],
                    rhs=Lsb[:, sub * 512:(sub + 1) * 512],
                    start=True, stop=True)
            rc = rcpool.tile([P, 1024], f32, tag="rc", name="rc")
            nc.vector.reciprocal_approx_fast(out=rc[:], in_=bc[:])
            nc.gpsimd.tensor_mul(
                AT_sb[0:64, pr, qc * 1024:(qc + 1) * 1024],
                atA[0:64, :], rc[0:64, :])
            nc.gpsimd.tensor_mul(
                AT_sb[64:128, pr, qc * 1024:(qc + 1) * 1024],
                atB[64:128, :], rc[64:128, :])

        blocks = [(qc, pr) for qc in range(2) for pr in range(NPC)]
        for bi, (qc, pr) in enumerate(blocks):
            esA = [None] * NKC
            esB = [None] * NKC
            accA = accB = None
            for kc in range(NKC):
                esA[kc], esB[kc] = emit_scores_exp(pr, qc, kc)
                if kc == 6 and qc == 0 and pr < 3:
                    qprefetch(pr + 1)
                if kc == 2:
                    accA = psAcA.tile([P, 1024], f32, tag="av", name="accA")
                    accB = psAcB.tile([P, 1024], f32, tag="av", name="accB")
                    emit_av(accA, 2 * pr + 0, esA[0], 0, True, False)
                    emit_av(accB, 2 * pr + 1, esB[0], 0, True, False)
                elif kc > 2:
                    emit_av(accA, 2 * pr + 0, esA[kc - 2], kc - 2, False, False)
                    emit_av(accB, 2 * pr + 1, esB[kc - 2], kc - 2, False, False)
            block_tails(pr, qc, accA, accB, esA, esB)

            # boundary work: trailing Q projections during qc0, qc0's output
            # projection during qc1 boundaries; qc1's C lands in the drain.
            tc.no_sync_barrier()
            if qc == 0 and pr < 3:
                emit_qproj(pr + 1)
            if bi >= 3:
                # 4 qc0 C-chunks at each of the last 5 boundaries handles
                # 16 chunks by the end of block (1, pr2); emit 4 per boundary
                # starting after block (0, pr3).
                cidx = (bi - 3) * 4
                if cidx < 16:
                    for j in range(cidx, cidx + 4):
                        so, oc = j // 2, j % 2
                        emit_c_chunk(so, oc, psS4[j % 4])

        # drain: qc1's output projection
        tc.no_sync_barrier()
        for j in range(16):
            so, oc = 8 + j // 2, j % 2
            emit_c_chunk(so, oc, psS4[j % 4])

    nc.compile()
    return nc


def _get_nc():
    if "nc" not in _cache:
        _cache["nc"] = _build()
    return _cache["nc"]


def make_in_maps(q, k, v, mask, Wq, bq, Wk, bk, Wv, bv, Wo, bo):
    """Host-side sharding: slice/transpose the full inputs per core."""
    import ml_dtypes
    f = np.float32
    bf = ml_dtypes.bfloat16
    q = np.asarray(q, dtype=f)
    k = np.asarray(k, dtype=f)
    v = np.asarray(v, dtype=f)
    Wq = np.asarray(Wq, dtype=f) * (L2E / 8.0)   # exp2-domain prescale
    Wk = np.asarray(Wk, dtype=f)
    Wv = np.asarray(Wv, dtype=f)
    Wo = np.asarray(Wo, dtype=f)
    bq = np.asarray(bq, dtype=f) * (L2E / 8.0)
    bk = np.asarray(bk, dtype=f)
    mask = np.asarray(mask)

    sel = np.zeros((P, 128), dtype=f)
    sel[0, 0:64] = 1.0
    sel[32, 64:128] = 1.0

    def act_perm(x):
        # [S, D] -> [NSC, P, NDM, 512]: chunk sc holds x^T[d, sc*512 + j]
        # with d = o*128 + p, laid out as one dense 8KB line per partition.
        return np.ascontiguousarray(
            x.reshape(NSC, 512, NDM, P).transpose(0, 3, 2, 1)).astype(bf)

    def w_perm(w):
        # [D, PROJ] -> [P, NDM, PROJ]
        return np.ascontiguousarray(
            w.reshape(NDM, P, -1).transpose(1, 0, 2)).astype(bf)

    in_maps = []
    for c in range(NCORES):
        b, hg = divmod(c, 2)
        cols = slice(hg * PROJ, (hg + 1) * PROJ)
        mvals = (mask[b, 0, 0, :] != 0).astype(f)            # [S]
        mk2 = np.ascontiguousarray(mvals.reshape(NSO, P).T)  # [P, NSO]
        in_maps.append({
            "qT": act_perm(q[b]),
            "kT": act_perm(k[b]),
            "vT": act_perm(v[b]),
            "wq": w_perm(Wq[:, cols]),
            "wk": w_perm(Wk[:, cols]),
            "wv": w_perm(Wv[:, cols]),
            "wo": np.ascontiguousarray(
                Wo[cols, :].reshape(NPC, P, D).transpose(1, 0, 2)).astype(bf),
            "bq2": np.ascontiguousarray(bq[cols].reshape(NPC, P).T),
            "bk2": np.ascontiguousarray(bk[cols].reshape(NPC, P).T),
            "mk": mk2,
            "sel": sel.astype(bf),
        })
    return in_maps


def combine_outputs(parts, Wv_bv_Wo_bo):
    """Sum the two head-group partials per batch, add bv @ Wo + bo."""
    bv, Wo, bo = Wv_bv_Wo_bo
    bo_eff = (np.asarray(bv, np.float32) @ np.asarray(Wo, np.float32)
              + np.asarray(bo, np.float32))
    out = np.empty((B, S, D), dtype=np.float32)
    for b in range(B):
        out[b] = parts[2 * b] + parts[2 * b + 1] + bo_eff
    return out


def _install_axon_ntff_hook():
    """The agent image's antenv lacks axon_hooks; synthesize it and register
    the ctypes NTFF profile hook from trn_boot so trace=True works."""
    import sys
    import types
    if "antenv.axon_hooks" in sys.modules:
        return
    try:
        from trn_agent_boot.trn_boot import _ntff_profile_via_ctypes
        hook = _ntff_profile_via_ctypes("/opt/axon/libaxon_pjrt.so")
    except Exception:
        hook = None
    mod = types.ModuleType("antenv.axon_hooks")
    mod._hook = hook
    mod.get_axon_ntff_profile_hook = lambda: mod._hook
    mod.set_axon_ntff_profile_hook = lambda h: setattr(mod, "_hook", h)
    sys.modules["antenv.axon_hooks"] = mod
    import concourse.bass_utils as bu
    bu.upload_artifacts = lambda tmpdir: str(tmpdir)


def kernel(q, k, v, mask, Wq, bq, Wk, bk, Wv, bv, Wo, bo):
    from concourse.bass_utils import run_bass_kernel_spmd

    nc = _get_nc()
    in_maps = make_in_maps(q, k, v, mask, Wq, bq, Wk, bk, Wv, bv, Wo, bo)
    trace = bool(int(os.environ.get("KERNEL_TRACE", "0")))
    if trace:
        try:
            _install_axon_ntff_hook()
        except Exception:
            trace = False
    try:
        res = run_bass_kernel_spmd(
            nc, in_maps, list(range(NCORES)), trace=trace,
            tmpdir=os.environ.get("KERNEL_TRACE_DIR") or None)
    except Exception:
        if not trace:
            raise
        res = run_bass_kernel_spmd(nc, in_maps, list(range(NCORES)), trace=False)
    _cache["last_result"] = res
    parts = [res.results[c]["out"] for c in range(NCORES)]
    return combine_outputs(parts, (bv, Wo, bo))


# revision 22
# speedup vs baseline: 1.6634x; 1.0628x over previous
"""Multi-head attention (B=4, S=2048, D=1024, H=16) on 8 Trainium2 cores.

Sharding: core c handles batch b = c//2 and head-group hg = c%2 (8 of the 16
heads, 512 of the 1024 projection dims).  Host sums the two head-group
partials per batch (the "all-reduce after w_o") and adds bv@Wo + bo.

v2 design (vs the 425us baseline):
  * Pair-blocks: per (qc, pr) block both heads' scores are computed with
    64-row PE tiling (K=64 row tiles run concurrently -> 2x score matmul
    throughput; probe-measured 110ns/MM vs 216 standard).
  * Q is pre-scaled by log2(e)/8 on the host (folded into Wq/bq) so scores
    are in the exp2 domain.  exp for head-even tiles runs on ACT
    (exp(y*ln2)); head-odd tiles mostly run on the DVE as a one-instruction
    Schraudolph exp2 (tensor_scalar mult+add -> int16 bits == bf16 exp2
    approximation, ~3.3% max elementwise, cancels through softmax to ~1e-2
    final).  This splits the 293us exp load across two engines.
  * PSUM: SA + SB score tiles + two AV accumulators = exactly 8 banks.
    The l (softmax denominator) rides the AV matmul as V's 65th column.
  * Phase A emits K, V, Q(pair0) only; Q(pair 1-3) projections trail at
    qc0 block boundaries, C chunks of qc0 at qc1 boundaries, C of qc1 in
    the drain.  1/l uses reciprocal_approx_fast (5x the DVE reciprocal).

All matmuls are bf16 with fp32 PSUM accumulation.
"""

import os
import numpy as np

B, S, D = 4, 2048, 1024
H, DK = 16, 64
P = 128
NCORES = 8
HPC = H // 2            # heads per core
PROJ = HPC * DK         # 512 projection dims per core
NDM = D // P            # 8 d_model chunks
NPC = PROJ // P         # 4 head-pair chunks
NSC = S // 512          # 4 seq chunks of 512
NSO = S // P            # 16 seq chunks of 128
NKC = S // P            # 16 key chunks of 128

L2E = float(np.log2(np.e))
LN2 = float(np.log(2.0))
C_SCH = 16250.5
# Every kc runs one exp tile on ACT and one on the DVE (Schraudolph), so the
# two engines stream in parallel.  kc in DVE_KCS -> head-odd tile on DVE;
# else head-even on DVE.  Alternating keeps the approximation error split
# evenly across both heads.  f_schraudolph = 0.5 structurally.
DVE_KCS = frozenset(range(0, NKC, 2))

_cache = {}


def _build():
    import concourse.bass as bass
    import concourse.bacc as bacc
    import concourse.mybir as mybir
    import concourse.tile as tile
    from contextlib import ExitStack

    f32 = mybir.dt.float32
    bf16 = mybir.dt.bfloat16
    i16 = mybir.dt.int16
    AF = mybir.ActivationFunctionType
    MUL = mybir.AluOpType.mult
    ADD = mybir.AluOpType.add

    nc = bacc.Bacc("TRN2", target_bir_lowering=False, debug=False,
                   num_devices=NCORES)

    # Activations/weights arrive pre-permuted from the host so every DMA is
    # a dense per-partition burst (8KB lines) instead of 1KB strided lines.
    qT = nc.dram_tensor("qT", [NSC, P, NDM, 512], bf16, kind="ExternalInput").ap()
    kT = nc.dram_tensor("kT", [NSC, P, NDM, 512], bf16, kind="ExternalInput").ap()
    vT = nc.dram_tensor("vT", [NSC, P, NDM, 512], bf16, kind="ExternalInput").ap()
    wq = nc.dram_tensor("wq", [P, NDM, PROJ], bf16, kind="ExternalInput").ap()
    wk = nc.dram_tensor("wk", [P, NDM, PROJ], bf16, kind="ExternalInput").ap()
    wv = nc.dram_tensor("wv", [P, NDM, PROJ], bf16, kind="ExternalInput").ap()
    wo = nc.dram_tensor("wo", [P, NPC, D], bf16, kind="ExternalInput").ap()
    bq2 = nc.dram_tensor("bq2", [P, NPC], f32, kind="ExternalInput").ap()
    bk2 = nc.dram_tensor("bk2", [P, NPC], f32, kind="ExternalInput").ap()
    mk = nc.dram_tensor("mk", [P, NSO], f32, kind="ExternalInput").ap()
    sel = nc.dram_tensor("sel", [P, 128], bf16, kind="ExternalInput").ap()
    out = nc.dram_tensor("out", [S, D], f32, kind="ExternalOutput").ap()

    with tile.TileContext(nc) as tc, ExitStack() as ctx:
        cpool = ctx.enter_context(tc.tile_pool(name="const", bufs=1))
        sel_sb = cpool.tile([P, 128], bf16)
        nc.sync.dma_start(sel_sb[:], sel)
        bq_sb = cpool.tile([P, NPC], f32)
        nc.sync.dma_start(bq_sb[:], bq2)
        bk_sb = cpool.tile([P, NPC], f32)
        nc.sync.dma_start(bk_sb[:], bk2)
        mk_sb = cpool.tile([P, NSO], f32)
        nc.sync.dma_start(mk_sb[:], mk)
        # l values land in rows {0, 32}; other rows must stay finite for the
        # sel broadcast matmul.
        Lsb = cpool.tile([P, 1024], bf16)
        nc.gpsimd.memset(Lsb[:], 0.0)
        ones8 = cpool.tile([P, HPC], bf16)
        nc.gpsimd.memset(ones8[:], 1.0)

        respool = ctx.enter_context(tc.tile_pool(name="res", bufs=1))
        # Q^T pair-stacked: rows 0-63 = head 2*pr dims, 64-127 = head
        # 2*pr+1.  K^T stored per head on the full 128-partition contraction
        # range (even heads rows 0-63, odd heads 64-127, rest zero) so the
        # score matmuls are full-array standard-mode matmuls: partial-array
        # tiling modes do not register as PE-busy in the HAM activity
        # monitor and leave the clock gate throttled at 1.2 GHz (measured:
        # 414us throttle-active with 64-row tiling vs 18us without).
        QT_sb = respool.tile([P, NPC, S], bf16)
        KT_sb = respool.tile([P, HPC, S], bf16)
        nc.vector.memset(KT_sb[:], 0.0)
        # V with an interleaved mask column per head: head h occupies cols
        # [h*65, h*65+64) and col h*65+64 == mask (the masked softmax
        # denominator rides the AV matmul as output partition 64).
        V_sb = respool.tile([P, NSO, HPC * (DK + 1)], bf16)
        for so in range(NSO):
            # mask columns filled on gpsimd so the DVE queue stays clear for
            # the K-projection bias adds.
            nc.gpsimd.tensor_scalar_mul(
                V_sb[:, so, :].rearrange("p (h w) -> p h w", w=DK + 1)[:, :, DK],
                ones8[:], mk_sb[:, so:so + 1])
        AT_sb = respool.tile([P, NPC, S], bf16)   # normalized A^T

        wopool = ctx.enter_context(tc.tile_pool(name="wo", bufs=1))
        wo_sb = wopool.tile([P, NPC, D], bf16)

        npool = ctx.enter_context(tc.tile_pool(name="norm", bufs=4))
        rcpool = ctx.enter_context(tc.tile_pool(name="rc", bufs=2))
        epA = ctx.enter_context(tc.tile_pool(name="expA", bufs=5))
        epB = ctx.enter_context(tc.tile_pool(name="expB", bufs=6))
        opool = ctx.enter_context(tc.tile_pool(name="ostage", bufs=4))

        # Weight + activation staging pools live for the whole kernel: wq is
        # needed for trailing Q projections inside phase B.
        wpool = ctx.enter_context(tc.tile_pool(name="w", bufs=2))
        apool = ctx.enter_context(tc.tile_pool(name="act", bufs=4))

        # ---------------- Phase A: K, V, Q(pair0) ----------------
        with ExitStack() as ctxA:
            psA = ctxA.enter_context(
                tc.tile_pool(name="psA", bufs=4, space="PSUM"))

            # K projection -> pair-packed KT_sb
            wk_sb = wpool.tile([P, NDM, PROJ], bf16, tag="w", name="wk_sb")
            nc.sync.dma_start(wk_sb[:], wk)
            for sc in range(NSC):
                a_sb = apool.tile([P, NDM, 512], bf16, tag="a", name="a_sb")
                nc.sync.dma_start(a_sb[:], kT[sc])
                for pc in range(NPC):
                    ps = psA.tile([P, 512], f32, tag="pp", name="psa")
                    for dc in range(NDM):
                        nc.tensor.matmul(
                            ps,
                            lhsT=wk_sb[:, dc, pc * P:(pc + 1) * P],
                            rhs=a_sb[:, dc, :],
                            start=(dc == 0), stop=(dc == NDM - 1))
                    for half in range(2):
                        lo = half * 64
                        nc.vector.tensor_scalar_add(
                            KT_sb[lo:lo + 64, 2 * pc + half,
                                  sc * 512:(sc + 1) * 512],
                            ps[lo:lo + 64, :],
                            bk_sb[lo:lo + 64, pc:pc + 1])

            # V projection (mask folded in)
            wv_sb = wpool.tile([P, NDM, PROJ], bf16, tag="w", name="wv_sb")
            nc.sync.dma_start(wv_sb[:], wv)
            for sc in range(NSC):
                a_sb = apool.tile([P, NDM, 512], bf16, tag="a", name="a_sb")
                nc.sync.dma_start(a_sb[:], vT[sc])
                for so4 in range(4):
                    so = sc * 4 + so4
                    ps = psA.tile([P, 512], f32, tag="pp", name="psa")
                    for dc in range(NDM):
                        nc.tensor.matmul(
                            ps,
                            lhsT=a_sb[:, dc, so4 * P:(so4 + 1) * P],
                            rhs=wv_sb[:, dc, :],
                            start=(dc == 0), stop=(dc == NDM - 1))
                    nc.vector.tensor_scalar_mul(
                        V_sb[:, so, :].rearrange(
                            "p (h w) -> p h w", w=DK + 1)[:, :, 0:DK],
                        ps.rearrange("p (h w) -> p h w", w=DK),
                        mk_sb[:, so:so + 1])

            # Q projection, pair 0 only; wq stays resident for pairs 1-3.
            wq_sb = wpool.tile([P, NDM, PROJ], bf16, tag="w", name="wq_sb")
            nc.sync.dma_start(wq_sb[:], wq)
            for sc in range(NSC):
                a_sb = apool.tile([P, NDM, 512], bf16, tag="a", name="a_sb")
                nc.sync.dma_start(a_sb[:], qT[sc])
                ps = psA.tile([P, 512], f32, tag="pp", name="psa")
                for dc in range(NDM):
                    nc.tensor.matmul(
                        ps,
                        lhsT=wq_sb[:, dc, 0:P],
                        rhs=a_sb[:, dc, :],
                        start=(dc == 0), stop=(dc == NDM - 1))
                nc.vector.tensor_scalar_add(
                    QT_sb[:, 0, sc * 512:(sc + 1) * 512], ps,
                    bq_sb[:, 0:1])
            nc.sync.dma_start(wo_sb[:], wo)

        # ---------------- Phase B ----------------
        # Four single-bank score pools: each 512-column half of each head's
        # score tile is its own PSUM tile, so the next kc's matmul into a
        # half only waits for the one exp instruction that read that half.
        psS4 = [ctx.enter_context(
            tc.tile_pool(name=f"psS{i}", bufs=1, space="PSUM"))
            for i in range(4)]
        psAcA = ctx.enter_context(tc.tile_pool(name="psAcA", bufs=1, space="PSUM"))
        psAcB = ctx.enter_context(tc.tile_pool(name="psAcB", bufs=1, space="PSUM"))

        qstage = {}

        def qprefetch(pc):
            """DMA the qT chunks for pair pc's trailing projection."""
            tiles = []
            for sc in range(NSC):
                a_sb = apool.tile([P, NDM, 512], bf16, tag="a", name="a_q")
                nc.sync.dma_start(a_sb[:], qT[sc])
                tiles.append(a_sb)
            qstage[pc] = tiles

        def emit_qproj(pc):
            """Trailing Q projection for pair pc (borrows psSA/psSB)."""
            tiles = qstage.pop(pc)
            for sc in range(NSC):
                a_sb = tiles[sc]
                pool = psS4[sc]
                ps = pool.tile([P, 512], f32, tag="s", name="ps_q")
                for dc in range(NDM):
                    nc.tensor.matmul(
                        ps,
                        lhsT=wq_sb[:, dc, pc * P:(pc + 1) * P],
                        rhs=a_sb[:, dc, :],
                        start=(dc == 0), stop=(dc == NDM - 1))
                nc.vector.tensor_scalar_add(
                    QT_sb[:, pc, sc * 512:(sc + 1) * 512], ps,
                    bq_sb[:, pc:pc + 1])

        def emit_c_chunk(so, oc, pool):
            """Output projection chunk [128q, 512o] (borrows a score pool)."""
            ps = pool.tile([P, 512], f32, tag="s", name="ps_c")
            for pc in range(NPC):
                nc.tensor.matmul(
                    ps,
                    lhsT=AT_sb[:, pc, so * P:(so + 1) * P],
                    rhs=wo_sb[:, pc, oc * 512:(oc + 1) * 512],
                    start=(pc == 0), stop=(pc == NPC - 1))
            ost = opool.tile([P, 512], f32, tag="o", name="ost")
            nc.scalar.copy(ost, ps)
            nc.sync.dma_start(
                out[so * P:(so + 1) * P, oc * 512:(oc + 1) * 512], ost)

        def emit_scores_exp(pr, qc, kc):
            """Standard-mode pair scores + the pair's exp, one tile/engine.

            Both engines consume their score halves as separate N=512
            instructions against separate single-bank PSUM tiles; the
            ACT-side tile is computed first since its chain is longest.
            """
            ea = epA.tile([P, 1024], bf16, tag="e", name="ea")
            eb = epB.tile([P, 1024], bf16, tag="e", name="eb")
            if kc in DVE_KCS:
                order = ((0, ea, True), (1, eb, False))
            else:
                order = ((1, eb, True), (0, ea, False))
            for hi, et, on_act in order:
                for sub in range(2):
                    ssl = slice(sub * 512, (sub + 1) * 512)
                    cols = slice(qc * 1024 + sub * 512,
                                 qc * 1024 + (sub + 1) * 512)
                    ps = psS4[2 * hi + sub].tile([P, 512], f32, tag="s",
                                                 name="ssc")
                    nc.tensor.matmul(
                        ps,
                        lhsT=KT_sb[:, 2 * pr + hi, kc * P:(kc + 1) * P],
                        rhs=QT_sb[:, pr, cols],
                        start=True, stop=True)
                    if on_act:
                        nc.scalar.activation(et[:, ssl], ps, AF.Exp,
                                             scale=LN2)
                    else:
                        nc.vector.tensor_scalar(
                            et[:, ssl].bitcast(i16), ps,
                            128.0, C_SCH, MUL, ADD)
            return ea, eb

        def emit_av(acc, h, e, kc, start, stop):
            for sub in range(2):
                nc.tensor.matmul(
                    acc[0:DK + 1, sub * 512:(sub + 1) * 512],
                    lhsT=V_sb[:, kc, h * (DK + 1):(h + 1) * (DK + 1)],
                    rhs=e[:, sub * 512:(sub + 1) * 512],
                    start=start, stop=stop)

        def tails_front(pr, qc, accA, accB, esA, esB):
            """Last AVs + PSUM evacuation (ACT does the big copies)."""
            emit_av(accA, 2 * pr + 0, esA[NKC - 2], NKC - 2, False, False)
            emit_av(accA, 2 * pr + 0, esA[NKC - 1], NKC - 1, False, True)
            emit_av(accB, 2 * pr + 1, esB[NKC - 2], NKC - 2, False, False)
            emit_av(accB, 2 * pr + 1, esB[NKC - 1], NKC - 1, False, True)
            nc.vector.tensor_copy(Lsb[0:1, :], accA[DK:DK + 1, :])
            nc.vector.tensor_copy(Lsb[32:33, :], accB[DK:DK + 1, :])
            atA = npool.tile([P, 1024], f32, tag="at", name="atA")
            nc.scalar.copy(atA[0:64, :], accA[0:64, :])
            atB = npool.tile([P, 1024], f32, tag="at", name="atB")
            nc.scalar.copy(atB[64:128, :], accB[0:64, :])
            return atA, atB

        def tails_back(pr, qc, atA, atB):
            """l broadcast (into accB's freed banks), 1/l, normalize.

            Runs after the boundary chunks so its serial ACT/DVE chain
            overlaps the chunks' matmuls; normalize on GPSIMD keeps the
            DVE free for the next block's exp stream.
            """
            bc = psAcB.tile([P, 1024], f32, tag="av", name="bc")
            for sub in range(2):
                nc.tensor.matmul(
                    bc[:, sub * 512:(sub + 1) * 512],
                    lhsT=sel_sb[:],
                    rhs=Lsb[:, sub * 512:(sub + 1) * 512],
                    start=True, stop=True)
            rc = rcpool.tile([P, 1024], f32, tag="rc", name="rc")
            nc.vector.reciprocal_approx_fast(out=rc[:], in_=bc[:])
            nc.gpsimd.tensor_mul(
                AT_sb[0:64, pr, qc * 1024:(qc + 1) * 1024],
                atA[0:64, :], rc[0:64, :])
            nc.gpsimd.tensor_mul(
                AT_sb[64:128, pr, qc * 1024:(qc + 1) * 1024],
                atB[64:128, :], rc[64:128, :])

        blocks = [(qc, pr) for qc in range(2) for pr in range(NPC)]
        for bi, (qc, pr) in enumerate(blocks):
            esA = [None] * NKC
            esB = [None] * NKC
            accA = accB = None
            for kc in range(NKC):
                esA[kc], esB[kc] = emit_scores_exp(pr, qc, kc)
                if kc == 6 and qc == 0 and pr < 3:
                    qprefetch(pr + 1)
                if kc == 2:
                    accA = psAcA.tile([P, 1024], f32, tag="av", name="accA")
                    accB = psAcB.tile([P, 1024], f32, tag="av", name="accB")
                    emit_av(accA, 2 * pr + 0, esA[0], 0, True, False)
                    emit_av(accB, 2 * pr + 1, esB[0], 0, True, False)
                elif kc > 2:
                    emit_av(accA, 2 * pr + 0, esA[kc - 2], kc - 2, False, False)
                    emit_av(accB, 2 * pr + 1, esB[kc - 2], kc - 2, False, False)
            atA, atB = tails_front(pr, qc, accA, accB, esA, esB)

            # boundary work between the evacuation and the bc/rc/normalize
            # chain, so the PE streams chunks while ACT/DVE drain the
            # accumulators: trailing Q projections during qc0, qc0's output
            # projection during qc1 boundaries; qc1's C lands in the drain.
            # bi==3 must normalize first (its own AT feeds the chunks) and
            # bi==7 is the drain, handled below.
            tc.no_sync_barrier()
            if qc == 0 and pr < 3:
                emit_qproj(pr + 1)
                tails_back(pr, qc, atA, atB)
            elif bi == 3:
                tails_back(pr, qc, atA, atB)
                tc.no_sync_barrier()
                for j in range(4):
                    emit_c_chunk(j // 2, j % 2, psS4[j % 4])
            elif bi < 7:
                cidx = (bi - 3) * 4
                for j in range(cidx, cidx + 4):
                    emit_c_chunk(j // 2, j % 2, psS4[j % 4])
                tails_back(pr, qc, atA, atB)

        # drain: the last block's normalize in halves (on the DVE -- it is
        # on the critical path here), each half releasing its 8 output
        # chunks of qc1's projection.
        bc = psAcB.tile([P, 1024], f32, tag="av", name="bc")
        for sub in range(2):
            nc.tensor.matmul(
                bc[:, sub * 512:(sub + 1) * 512],
                lhsT=sel_sb[:],
                rhs=Lsb[:, sub * 512:(sub + 1) * 512],
                start=True, stop=True)
        rc = rcpool.tile([P, 1024], f32, tag="rc", name="rc")
        nc.vector.reciprocal_approx_fast(out=rc[:], in_=bc[:])
        for half in range(2):
            hsl = slice(half * 512, (half + 1) * 512)
            nc.vector.tensor_tensor(
                AT_sb[0:64, 3, 1024 + half * 512:1024 + (half + 1) * 512],
                atA[0:64, hsl], rc[0:64, hsl], MUL)
            nc.vector.tensor_tensor(
                AT_sb[64:128, 3, 1024 + half * 512:1024 + (half + 1) * 512],
                atB[64:128, hsl], rc[64:128, hsl], MUL)
            tc.no_sync_barrier()
            for j in range(8):
                so, oc = 8 + half * 4 + j // 2, j % 2
                emit_c_chunk(so, oc, psS4[j % 4])

    nc.compile()
    return nc


def _get_nc():
    if "nc" not in _cache:
        _cache["nc"] = _build()
    return _cache["nc"]


def make_in_maps(q, k, v, mask, Wq, bq, Wk, bk, Wv, bv, Wo, bo):
    """Host-side sharding: slice/transpose the full inputs per core."""
    import ml_dtypes
    f = np.float32
    bf = ml_dtypes.bfloat16
    q = np.asarray(q, dtype=f)
    k = np.asarray(k, dtype=f)
    v = np.asarray(v, dtype=f)
    Wq = np.asarray(Wq, dtype=f) * (L2E / 8.0)   # exp2-domain prescale
    Wk = np.asarray(Wk, dtype=f)
    Wv = np.asarray(Wv, dtype=f)
    Wo = np.asarray(Wo, dtype=f)
    bq = np.asarray(bq, dtype=f) * (L2E / 8.0)
    bk = np.asarray(bk, dtype=f)
    mask = np.asarray(mask)

    sel = np.zeros((P, 128), dtype=f)
    sel[0, 0:64] = 1.0
    sel[32, 64:128] = 1.0

    def act_perm(x):
        # [S, D] -> [NSC, P, NDM, 512]: chunk sc holds x^T[d, sc*512 + j]
        # with d = o*128 + p, laid out as one dense 8KB line per partition.
        return np.ascontiguousarray(
            x.reshape(NSC, 512, NDM, P).transpose(0, 3, 2, 1)).astype(bf)

    def w_perm(w):
        # [D, PROJ] -> [P, NDM, PROJ]
        return np.ascontiguousarray(
            w.reshape(NDM, P, -1).transpose(1, 0, 2)).astype(bf)

    in_maps = []
    for c in range(NCORES):
        b, hg = divmod(c, 2)
        cols = slice(hg * PROJ, (hg + 1) * PROJ)
        mvals = (mask[b, 0, 0, :] != 0).astype(f)            # [S]
        mk2 = np.ascontiguousarray(mvals.reshape(NSO, P).T)  # [P, NSO]
        in_maps.append({
            "qT": act_perm(q[b]),
            "kT": act_perm(k[b]),
            "vT": act_perm(v[b]),
            "wq": w_perm(Wq[:, cols]),
            "wk": w_perm(Wk[:, cols]),
            "wv": w_perm(Wv[:, cols]),
            "wo": np.ascontiguousarray(
                Wo[cols, :].reshape(NPC, P, D).transpose(1, 0, 2)).astype(bf),
            "bq2": np.ascontiguousarray(bq[cols].reshape(NPC, P).T),
            "bk2": np.ascontiguousarray(bk[cols].reshape(NPC, P).T),
            "mk": mk2,
            "sel": sel.astype(bf),
        })
    return in_maps


def combine_outputs(parts, Wv_bv_Wo_bo):
    """Sum the two head-group partials per batch, add bv @ Wo + bo."""
    bv, Wo, bo = Wv_bv_Wo_bo
    bo_eff = (np.asarray(bv, np.float32) @ np.asarray(Wo, np.float32)
              + np.asarray(bo, np.float32))
    out = np.empty((B, S, D), dtype=np.float32)
    for b in range(B):
        out[b] = parts[2 * b] + parts[2 * b + 1] + bo_eff
    return out


def _install_axon_ntff_hook():
    """The agent image's antenv lacks axon_hooks; synthesize it and register
    the ctypes NTFF profile hook from trn_boot so trace=True works."""
    import sys
    import types
    if "antenv.axon_hooks" in sys.modules:
        return
    try:
        from trn_agent_boot.trn_boot import _ntff_profile_via_ctypes
        hook = _ntff_profile_via_ctypes("/opt/axon/libaxon_pjrt.so")
    except Exception:
        hook = None
    mod = types.ModuleType("antenv.axon_hooks")
    mod._hook = hook
    mod.get_axon_ntff_profile_hook = lambda: mod._hook
    mod.set_axon_ntff_profile_hook = lambda h: setattr(mod, "_hook", h)
    sys.modules["antenv.axon_hooks"] = mod
    import concourse.bass_utils as bu
    bu.upload_artifacts = lambda tmpdir: str(tmpdir)


def kernel(q, k, v, mask, Wq, bq, Wk, bk, Wv, bv, Wo, bo):
    from concourse.bass_utils import run_bass_kernel_spmd

    nc = _get_nc()
    in_maps = make_in_maps(q, k, v, mask, Wq, bq, Wk, bk, Wv, bv, Wo, bo)
    trace = bool(int(os.environ.get("KERNEL_TRACE", "0")))
    if trace:
        try:
            _install_axon_ntff_hook()
        except Exception:
            trace = False
    try:
        res = run_bass_kernel_spmd(
            nc, in_maps, list(range(NCORES)), trace=trace,
            tmpdir=os.environ.get("KERNEL_TRACE_DIR") or None)
    except Exception:
        if not trace:
            raise
        res = run_bass_kernel_spmd(nc, in_maps, list(range(NCORES)), trace=False)
    _cache["last_result"] = res
    parts = [res.results[c]["out"] for c in range(NCORES)]
    return combine_outputs(parts, (bv, Wo, bo))


# revision 23
# speedup vs baseline: 1.6834x; 1.0120x over previous
"""Multi-head attention (B=4, S=2048, D=1024, H=16) on 8 Trainium2 cores.

Sharding: core c handles batch b = c//2 and head-group hg = c%2 (8 of the 16
heads, 512 of the 1024 projection dims).  Host sums the two head-group
partials per batch (the "all-reduce after w_o") and adds bv@Wo + bo.

v2 design (vs the 425us baseline):
  * Pair-blocks: per (qc, pr) block both heads' scores are computed with
    64-row PE tiling (K=64 row tiles run concurrently -> 2x score matmul
    throughput; probe-measured 110ns/MM vs 216 standard).
  * Q is pre-scaled by log2(e)/8 on the host (folded into Wq/bq) so scores
    are in the exp2 domain.  exp for head-even tiles runs on ACT
    (exp(y*ln2)); head-odd tiles mostly run on the DVE as a one-instruction
    Schraudolph exp2 (tensor_scalar mult+add -> int16 bits == bf16 exp2
    approximation, ~3.3% max elementwise, cancels through softmax to ~1e-2
    final).  This splits the 293us exp load across two engines.
  * PSUM: SA + SB score tiles + two AV accumulators = exactly 8 banks.
    The l (softmax denominator) rides the AV matmul as V's 65th column.
  * Phase A emits K, V, Q(pair0) only; Q(pair 1-3) projections trail at
    qc0 block boundaries, C chunks of qc0 at qc1 boundaries, C of qc1 in
    the drain.  1/l uses reciprocal_approx_fast (5x the DVE reciprocal).

All matmuls are bf16 with fp32 PSUM accumulation.
"""

import os
import numpy as np

B, S, D = 4, 2048, 1024
H, DK = 16, 64
P = 128
NCORES = 8
HPC = H // 2            # heads per core
PROJ = HPC * DK         # 512 projection dims per core
NDM = D // P            # 8 d_model chunks
NPC = PROJ // P         # 4 head-pair chunks
NSC = S // 512          # 4 seq chunks of 512
NSO = S // P            # 16 seq chunks of 128
NKC = S // P            # 16 key chunks of 128

L2E = float(np.log2(np.e))
LN2 = float(np.log(2.0))
C_SCH = 16250.5
# Every kc runs one exp tile on ACT and one on the DVE (Schraudolph), so the
# two engines stream in parallel.  kc in DVE_KCS -> head-odd tile on DVE;
# else head-even on DVE.  Alternating keeps the approximation error split
# evenly across both heads.  f_schraudolph = 0.5 structurally.
DVE_KCS = frozenset(range(0, NKC, 2))

_cache = {}


def _build():
    import concourse.bass as bass
    import concourse.bacc as bacc
    import concourse.mybir as mybir
    import concourse.tile as tile
    from contextlib import ExitStack

    f32 = mybir.dt.float32
    bf16 = mybir.dt.bfloat16
    i16 = mybir.dt.int16
    AF = mybir.ActivationFunctionType
    MUL = mybir.AluOpType.mult
    ADD = mybir.AluOpType.add

    nc = bacc.Bacc("TRN2", target_bir_lowering=False, debug=False,
                   num_devices=NCORES)

    # Activations/weights arrive pre-permuted from the host so every DMA is
    # a dense per-partition burst (8KB lines) instead of 1KB strided lines.
    qT = nc.dram_tensor("qT", [NSC, P, NDM, 512], bf16, kind="ExternalInput").ap()
    kT = nc.dram_tensor("kT", [NSC, P, NDM, 512], bf16, kind="ExternalInput").ap()
    vT = nc.dram_tensor("vT", [NSC, P, NDM, 512], bf16, kind="ExternalInput").ap()
    wq = nc.dram_tensor("wq", [P, NDM, PROJ], bf16, kind="ExternalInput").ap()
    wk = nc.dram_tensor("wk", [P, NDM, PROJ], bf16, kind="ExternalInput").ap()
    wv = nc.dram_tensor("wv", [P, NDM, PROJ], bf16, kind="ExternalInput").ap()
    wo = nc.dram_tensor("wo", [P, NPC, D], bf16, kind="ExternalInput").ap()
    bq2 = nc.dram_tensor("bq2", [P, NPC], f32, kind="ExternalInput").ap()
    bk2 = nc.dram_tensor("bk2", [P, NPC], f32, kind="ExternalInput").ap()
    mk = nc.dram_tensor("mk", [P, NSO], f32, kind="ExternalInput").ap()
    sel = nc.dram_tensor("sel", [P, 128], bf16, kind="ExternalInput").ap()
    f16 = mybir.dt.float16
    out = nc.dram_tensor("out", [S, D], f16, kind="ExternalOutput").ap()

    with tile.TileContext(nc) as tc, ExitStack() as ctx:
        cpool = ctx.enter_context(tc.tile_pool(name="const", bufs=1))
        sel_sb = cpool.tile([P, 128], bf16)
        nc.sync.dma_start(sel_sb[:], sel)
        bq_sb = cpool.tile([P, NPC], f32)
        nc.sync.dma_start(bq_sb[:], bq2)
        bk_sb = cpool.tile([P, NPC], f32)
        nc.sync.dma_start(bk_sb[:], bk2)
        mk_sb = cpool.tile([P, NSO], f32)
        nc.sync.dma_start(mk_sb[:], mk)
        # l values land in rows {0, 32}; other rows must stay finite for the
        # sel broadcast matmul.
        Lsb = cpool.tile([P, 1024], bf16)
        nc.gpsimd.memset(Lsb[:], 0.0)
        ones8 = cpool.tile([P, HPC], bf16)
        nc.gpsimd.memset(ones8[:], 1.0)

        respool = ctx.enter_context(tc.tile_pool(name="res", bufs=1))
        # Q^T pair-stacked: rows 0-63 = head 2*pr dims, 64-127 = head
        # 2*pr+1.  K^T stored per head on the full 128-partition contraction
        # range (even heads rows 0-63, odd heads 64-127, rest zero) so the
        # score matmuls are full-array standard-mode matmuls: partial-array
        # tiling modes do not register as PE-busy in the HAM activity
        # monitor and leave the clock gate throttled at 1.2 GHz (measured:
        # 414us throttle-active with 64-row tiling vs 18us without).
        QT_sb = respool.tile([P, NPC, S], bf16)
        KT_sb = respool.tile([P, HPC, S], bf16)
        nc.vector.memset(KT_sb[:], 0.0)
        # V with an interleaved mask column per head: head h occupies cols
        # [h*65, h*65+64) and col h*65+64 == mask (the masked softmax
        # denominator rides the AV matmul as output partition 64).
        V_sb = respool.tile([P, NSO, HPC * (DK + 1)], bf16)
        for so in range(NSO):
            # mask columns filled on gpsimd so the DVE queue stays clear for
            # the K-projection bias adds.
            nc.gpsimd.tensor_scalar_mul(
                V_sb[:, so, :].rearrange("p (h w) -> p h w", w=DK + 1)[:, :, DK],
                ones8[:], mk_sb[:, so:so + 1])
        AT_sb = respool.tile([P, NPC, S], bf16)   # normalized A^T

        wopool = ctx.enter_context(tc.tile_pool(name="wo", bufs=1))
        wo_sb = wopool.tile([P, NPC, D], bf16)

        npool = ctx.enter_context(tc.tile_pool(name="norm", bufs=4))
        rcpool = ctx.enter_context(tc.tile_pool(name="rc", bufs=2))
        epA = ctx.enter_context(tc.tile_pool(name="expA", bufs=5))
        epB = ctx.enter_context(tc.tile_pool(name="expB", bufs=6))
        opool = ctx.enter_context(tc.tile_pool(name="ostage", bufs=4))

        # Weight + activation staging pools live for the whole kernel: wq is
        # needed for trailing Q projections inside phase B.
        wpool = ctx.enter_context(tc.tile_pool(name="w", bufs=2))
        apool = ctx.enter_context(tc.tile_pool(name="act", bufs=4))

        # ---------------- Phase A: K, V, Q(pair0) ----------------
        with ExitStack() as ctxA:
            psA = ctxA.enter_context(
                tc.tile_pool(name="psA", bufs=4, space="PSUM"))

            # K projection -> pair-packed KT_sb
            wk_sb = wpool.tile([P, NDM, PROJ], bf16, tag="w", name="wk_sb")
            nc.sync.dma_start(wk_sb[:], wk)
            for sc in range(NSC):
                a_sb = apool.tile([P, NDM, 512], bf16, tag="a", name="a_sb")
                nc.sync.dma_start(a_sb[:], kT[sc])
                for pc in range(NPC):
                    ps = psA.tile([P, 512], f32, tag="pp", name="psa")
                    for dc in range(NDM):
                        nc.tensor.matmul(
                            ps,
                            lhsT=wk_sb[:, dc, pc * P:(pc + 1) * P],
                            rhs=a_sb[:, dc, :],
                            start=(dc == 0), stop=(dc == NDM - 1))
                    for half in range(2):
                        lo = half * 64
                        nc.vector.tensor_scalar_add(
                            KT_sb[lo:lo + 64, 2 * pc + half,
                                  sc * 512:(sc + 1) * 512],
                            ps[lo:lo + 64, :],
                            bk_sb[lo:lo + 64, pc:pc + 1])

            # V projection (mask folded in)
            wv_sb = wpool.tile([P, NDM, PROJ], bf16, tag="w", name="wv_sb")
            nc.sync.dma_start(wv_sb[:], wv)
            for sc in range(NSC):
                a_sb = apool.tile([P, NDM, 512], bf16, tag="a", name="a_sb")
                nc.sync.dma_start(a_sb[:], vT[sc])
                for so4 in range(4):
                    so = sc * 4 + so4
                    ps = psA.tile([P, 512], f32, tag="pp", name="psa")
                    for dc in range(NDM):
                        nc.tensor.matmul(
                            ps,
                            lhsT=a_sb[:, dc, so4 * P:(so4 + 1) * P],
                            rhs=wv_sb[:, dc, :],
                            start=(dc == 0), stop=(dc == NDM - 1))
                    nc.vector.tensor_scalar_mul(
                        V_sb[:, so, :].rearrange(
                            "p (h w) -> p h w", w=DK + 1)[:, :, 0:DK],
                        ps.rearrange("p (h w) -> p h w", w=DK),
                        mk_sb[:, so:so + 1])

            # Q projection, pair 0 only; wq stays resident for pairs 1-3.
            wq_sb = wpool.tile([P, NDM, PROJ], bf16, tag="w", name="wq_sb")
            nc.sync.dma_start(wq_sb[:], wq)
            for sc in range(NSC):
                a_sb = apool.tile([P, NDM, 512], bf16, tag="a", name="a_sb")
                nc.sync.dma_start(a_sb[:], qT[sc])
                ps = psA.tile([P, 512], f32, tag="pp", name="psa")
                for dc in range(NDM):
                    nc.tensor.matmul(
                        ps,
                        lhsT=wq_sb[:, dc, 0:P],
                        rhs=a_sb[:, dc, :],
                        start=(dc == 0), stop=(dc == NDM - 1))
                nc.vector.tensor_scalar_add(
                    QT_sb[:, 0, sc * 512:(sc + 1) * 512], ps,
                    bq_sb[:, 0:1])
            nc.sync.dma_start(wo_sb[:], wo)

        # ---------------- Phase B ----------------
        # Four single-bank score pools: each 512-column half of each head's
        # score tile is its own PSUM tile, so the next kc's matmul into a
        # half only waits for the one exp instruction that read that half.
        psS4 = [ctx.enter_context(
            tc.tile_pool(name=f"psS{i}", bufs=1, space="PSUM"))
            for i in range(4)]
        psAcA = ctx.enter_context(tc.tile_pool(name="psAcA", bufs=1, space="PSUM"))
        psAcB = ctx.enter_context(tc.tile_pool(name="psAcB", bufs=1, space="PSUM"))

        qstage = {}

        def qprefetch(pc):
            """DMA the qT chunks for pair pc's trailing projection."""
            tiles = []
            for sc in range(NSC):
                a_sb = apool.tile([P, NDM, 512], bf16, tag="a", name="a_q")
                nc.sync.dma_start(a_sb[:], qT[sc])
                tiles.append(a_sb)
            qstage[pc] = tiles

        def emit_qproj(pc):
            """Trailing Q projection for pair pc (borrows psSA/psSB)."""
            tiles = qstage.pop(pc)
            for sc in range(NSC):
                a_sb = tiles[sc]
                pool = psS4[sc]
                ps = pool.tile([P, 512], f32, tag="s", name="ps_q")
                for dc in range(NDM):
                    nc.tensor.matmul(
                        ps,
                        lhsT=wq_sb[:, dc, pc * P:(pc + 1) * P],
                        rhs=a_sb[:, dc, :],
                        start=(dc == 0), stop=(dc == NDM - 1))
                nc.vector.tensor_scalar_add(
                    QT_sb[:, pc, sc * 512:(sc + 1) * 512], ps,
                    bq_sb[:, pc:pc + 1])

        def emit_c_chunk(so, oc, pool):
            """Output projection chunk [128q, 512o] (borrows a score pool)."""
            ps = pool.tile([P, 512], f32, tag="s", name="ps_c")
            for pc in range(NPC):
                nc.tensor.matmul(
                    ps,
                    lhsT=AT_sb[:, pc, so * P:(so + 1) * P],
                    rhs=wo_sb[:, pc, oc * 512:(oc + 1) * 512],
                    start=(pc == 0), stop=(pc == NPC - 1))
            ost = opool.tile([P, 512], f16, tag="o", name="ost")
            nc.scalar.copy(ost, ps)
            nc.sync.dma_start(
                out[so * P:(so + 1) * P, oc * 512:(oc + 1) * 512], ost)

        def emit_scores_exp(pr, qc, kc):
            """Standard-mode pair scores + the pair's exp, one tile/engine.

            Both engines consume their score halves as separate N=512
            instructions against separate single-bank PSUM tiles; the
            ACT-side tile is computed first since its chain is longest.
            """
            ea = epA.tile([P, 1024], bf16, tag="e", name="ea")
            eb = epB.tile([P, 1024], bf16, tag="e", name="eb")
            if kc in DVE_KCS:
                order = ((0, ea, True), (1, eb, False))
            else:
                order = ((1, eb, True), (0, ea, False))
            for hi, et, on_act in order:
                for sub in range(2):
                    ssl = slice(sub * 512, (sub + 1) * 512)
                    cols = slice(qc * 1024 + sub * 512,
                                 qc * 1024 + (sub + 1) * 512)
                    ps = psS4[2 * hi + sub].tile([P, 512], f32, tag="s",
                                                 name="ssc")
                    nc.tensor.matmul(
                        ps,
                        lhsT=KT_sb[:, 2 * pr + hi, kc * P:(kc + 1) * P],
                        rhs=QT_sb[:, pr, cols],
                        start=True, stop=True)
                    if on_act:
                        nc.scalar.activation(et[:, ssl], ps, AF.Exp,
                                             scale=LN2)
                    else:
                        nc.vector.tensor_scalar(
                            et[:, ssl].bitcast(i16), ps,
                            128.0, C_SCH, MUL, ADD)
            return ea, eb

        def emit_av(acc, h, e, kc, start, stop):
            for sub in range(2):
                nc.tensor.matmul(
                    acc[0:DK + 1, sub * 512:(sub + 1) * 512],
                    lhsT=V_sb[:, kc, h * (DK + 1):(h + 1) * (DK + 1)],
                    rhs=e[:, sub * 512:(sub + 1) * 512],
                    start=start, stop=stop)

        def tails_front(pr, qc, accA, accB, esA, esB):
            """Last AVs + PSUM evacuation (ACT does the big copies)."""
            emit_av(accA, 2 * pr + 0, esA[NKC - 2], NKC - 2, False, False)
            emit_av(accA, 2 * pr + 0, esA[NKC - 1], NKC - 1, False, True)
            emit_av(accB, 2 * pr + 1, esB[NKC - 2], NKC - 2, False, False)
            emit_av(accB, 2 * pr + 1, esB[NKC - 1], NKC - 1, False, True)
            nc.vector.tensor_copy(Lsb[0:1, :], accA[DK:DK + 1, :])
            nc.vector.tensor_copy(Lsb[32:33, :], accB[DK:DK + 1, :])
            atA = npool.tile([P, 1024], f32, tag="at", name="atA")
            nc.scalar.copy(atA[0:64, :], accA[0:64, :])
            atB = npool.tile([P, 1024], f32, tag="at", name="atB")
            nc.scalar.copy(atB[64:128, :], accB[0:64, :])
            return atA, atB

        def tails_back(pr, qc, atA, atB):
            """l broadcast (into accB's freed banks), 1/l, normalize.

            Runs after the boundary chunks so its serial ACT/DVE chain
            overlaps the chunks' matmuls; normalize on GPSIMD keeps the
            DVE free for the next block's exp stream.
            """
            bc = psAcB.tile([P, 1024], f32, tag="av", name="bc")
            for sub in range(2):
                nc.tensor.matmul(
                    bc[:, sub * 512:(sub + 1) * 512],
                    lhsT=sel_sb[:],
                    rhs=Lsb[:, sub * 512:(sub + 1) * 512],
                    start=True, stop=True)
            rc = rcpool.tile([P, 1024], f32, tag="rc", name="rc")
            nc.vector.reciprocal_approx_fast(out=rc[:], in_=bc[:])
            nc.gpsimd.tensor_mul(
                AT_sb[0:64, pr, qc * 1024:(qc + 1) * 1024],
                atA[0:64, :], rc[0:64, :])
            nc.gpsimd.tensor_mul(
                AT_sb[64:128, pr, qc * 1024:(qc + 1) * 1024],
                atB[64:128, :], rc[64:128, :])

        blocks = [(qc, pr) for qc in range(2) for pr in range(NPC)]
        for bi, (qc, pr) in enumerate(blocks):
            esA = [None] * NKC
            esB = [None] * NKC
            accA = accB = None
            for kc in range(NKC):
                esA[kc], esB[kc] = emit_scores_exp(pr, qc, kc)
                if kc == 6 and qc == 0 and pr < 3:
                    qprefetch(pr + 1)
                if kc == 2:
                    accA = psAcA.tile([P, 1024], f32, tag="av", name="accA")
                    accB = psAcB.tile([P, 1024], f32, tag="av", name="accB")
                    emit_av(accA, 2 * pr + 0, esA[0], 0, True, False)
                    emit_av(accB, 2 * pr + 1, esB[0], 0, True, False)
                elif kc > 2:
                    emit_av(accA, 2 * pr + 0, esA[kc - 2], kc - 2, False, False)
                    emit_av(accB, 2 * pr + 1, esB[kc - 2], kc - 2, False, False)
            atA, atB = tails_front(pr, qc, accA, accB, esA, esB)

            # boundary work between the evacuation and the bc/rc/normalize
            # chain, so the PE streams chunks while ACT/DVE drain the
            # accumulators: trailing Q projections during qc0, qc0's output
            # projection during qc1 boundaries; qc1's C lands in the drain.
            # bi==3 must normalize first (its own AT feeds the chunks) and
            # bi==7 is the drain, handled below.
            tc.no_sync_barrier()
            if qc == 0 and pr < 3:
                emit_qproj(pr + 1)
                tails_back(pr, qc, atA, atB)
            elif bi == 3:
                # this block's own AT feeds the chunks: normalize the first
                # q-half on the DVE (critical path), release the chunks that
                # only read it, finish the second half on gpsimd behind them.
                bc = psAcB.tile([P, 1024], f32, tag="av", name="bc")
                for sub in range(2):
                    nc.tensor.matmul(
                        bc[:, sub * 512:(sub + 1) * 512],
                        lhsT=sel_sb[:],
                        rhs=Lsb[:, sub * 512:(sub + 1) * 512],
                        start=True, stop=True)
                rc = rcpool.tile([P, 1024], f32, tag="rc", name="rc")
                nc.vector.reciprocal_approx_fast(out=rc[:], in_=bc[:])
                nc.vector.tensor_tensor(
                    AT_sb[0:64, pr, 0:512], atA[0:64, 0:512],
                    rc[0:64, 0:512], MUL)
                nc.vector.tensor_tensor(
                    AT_sb[64:128, pr, 0:512], atB[64:128, 0:512],
                    rc[64:128, 0:512], MUL)
                tc.no_sync_barrier()
                for j in range(4):
                    emit_c_chunk(j // 2, j % 2, psS4[j % 4])
                nc.gpsimd.tensor_mul(
                    AT_sb[0:64, pr, 512:1024], atA[0:64, 512:1024],
                    rc[0:64, 512:1024])
                nc.gpsimd.tensor_mul(
                    AT_sb[64:128, pr, 512:1024], atB[64:128, 512:1024],
                    rc[64:128, 512:1024])
            elif bi < 7:
                cidx = (bi - 3) * 4
                for j in range(cidx, cidx + 4):
                    emit_c_chunk(j // 2, j % 2, psS4[j % 4])
                tails_back(pr, qc, atA, atB)

        # drain: the last block's normalize in halves (on the DVE -- it is
        # on the critical path here), each half releasing its 8 output
        # chunks of qc1's projection.
        bc = psAcB.tile([P, 1024], f32, tag="av", name="bc")
        for sub in range(2):
            nc.tensor.matmul(
                bc[:, sub * 512:(sub + 1) * 512],
                lhsT=sel_sb[:],
                rhs=Lsb[:, sub * 512:(sub + 1) * 512],
                start=True, stop=True)
        rc = rcpool.tile([P, 1024], f32, tag="rc", name="rc")
        nc.vector.reciprocal_approx_fast(out=rc[:], in_=bc[:])
        for qt in range(4):
            hsl = slice(qt * 256, (qt + 1) * 256)
            nc.vector.tensor_tensor(
                AT_sb[0:64, 3, 1024 + qt * 256:1024 + (qt + 1) * 256],
                atA[0:64, hsl], rc[0:64, hsl], MUL)
            nc.vector.tensor_tensor(
                AT_sb[64:128, 3, 1024 + qt * 256:1024 + (qt + 1) * 256],
                atB[64:128, hsl], rc[64:128, hsl], MUL)
            tc.no_sync_barrier()
            for j in range(4):
                so, oc = 8 + qt * 2 + j // 2, j % 2
                emit_c_chunk(so, oc, psS4[j % 4])

    nc.compile()
    return nc


def _get_nc():
    if "nc" not in _cache:
        _cache["nc"] = _build()
    return _cache["nc"]


def make_in_maps(q, k, v, mask, Wq, bq, Wk, bk, Wv, bv, Wo, bo):
    """Host-side sharding: slice/transpose the full inputs per core."""
    import ml_dtypes
    f = np.float32
    bf = ml_dtypes.bfloat16
    q = np.asarray(q, dtype=f)
    k = np.asarray(k, dtype=f)
    v = np.asarray(v, dtype=f)
    Wq = np.asarray(Wq, dtype=f) * (L2E / 8.0)   # exp2-domain prescale
    Wk = np.asarray(Wk, dtype=f)
    Wv = np.asarray(Wv, dtype=f)
    Wo = np.asarray(Wo, dtype=f)
    bq = np.asarray(bq, dtype=f) * (L2E / 8.0)
    bk = np.asarray(bk, dtype=f)
    mask = np.asarray(mask)

    sel = np.zeros((P, 128), dtype=f)
    sel[0, 0:64] = 1.0
    sel[32, 64:128] = 1.0

    def act_perm(x):
        # [S, D] -> [NSC, P, NDM, 512]: chunk sc holds x^T[d, sc*512 + j]
        # with d = o*128 + p, laid out as one dense 8KB line per partition.
        return np.ascontiguousarray(
            x.reshape(NSC, 512, NDM, P).transpose(0, 3, 2, 1)).astype(bf)

    def w_perm(w):
        # [D, PROJ] -> [P, NDM, PROJ]
        return np.ascontiguousarray(
            w.reshape(NDM, P, -1).transpose(1, 0, 2)).astype(bf)

    in_maps = []
    for c in range(NCORES):
        b, hg = divmod(c, 2)
        cols = slice(hg * PROJ, (hg + 1) * PROJ)
        mvals = (mask[b, 0, 0, :] != 0).astype(f)            # [S]
        mk2 = np.ascontiguousarray(mvals.reshape(NSO, P).T)  # [P, NSO]
        in_maps.append({
            "qT": act_perm(q[b]),
            "kT": act_perm(k[b]),
            "vT": act_perm(v[b]),
            "wq": w_perm(Wq[:, cols]),
            "wk": w_perm(Wk[:, cols]),
            "wv": w_perm(Wv[:, cols]),
            "wo": np.ascontiguousarray(
                Wo[cols, :].reshape(NPC, P, D).transpose(1, 0, 2)).astype(bf),
            "bq2": np.ascontiguousarray(bq[cols].reshape(NPC, P).T),
            "bk2": np.ascontiguousarray(bk[cols].reshape(NPC, P).T),
            "mk": mk2,
            "sel": sel.astype(bf),
        })
    return in_maps


def combine_outputs(parts, Wv_bv_Wo_bo):
    """Sum the two head-group partials per batch, add bv @ Wo + bo."""
    bv, Wo, bo = Wv_bv_Wo_bo
    bo_eff = (np.asarray(bv, np.float32) @ np.asarray(Wo, np.float32)
              + np.asarray(bo, np.float32))
    out = np.empty((B, S, D), dtype=np.float32)
    for b in range(B):
        out[b] = parts[2 * b] + parts[2 * b + 1] + bo_eff
    return out


def _install_axon_ntff_hook():
    """The agent image's antenv lacks axon_hooks; synthesize it and register
    the ctypes NTFF profile hook from trn_boot so trace=True works."""
    import sys
    import types
    if "antenv.axon_hooks" in sys.modules:
        return
    try:
        from trn_agent_boot.trn_boot import _ntff_profile_via_ctypes
        hook = _ntff_profile_via_ctypes("/opt/axon/libaxon_pjrt.so")
    except Exception:
        hook = None
    mod = types.ModuleType("antenv.axon_hooks")
    mod._hook = hook
    mod.get_axon_ntff_profile_hook = lambda: mod._hook
    mod.set_axon_ntff_profile_hook = lambda h: setattr(mod, "_hook", h)
    sys.modules["antenv.axon_hooks"] = mod
    import concourse.bass_utils as bu
    bu.upload_artifacts = lambda tmpdir: str(tmpdir)


def kernel(q, k, v, mask, Wq, bq, Wk, bk, Wv, bv, Wo, bo):
    from concourse.bass_utils import run_bass_kernel_spmd

    nc = _get_nc()
    in_maps = make_in_maps(q, k, v, mask, Wq, bq, Wk, bk, Wv, bv, Wo, bo)
    trace = bool(int(os.environ.get("KERNEL_TRACE", "0")))
    if trace:
        try:
            _install_axon_ntff_hook()
        except Exception:
            trace = False
    try:
        res = run_bass_kernel_spmd(
            nc, in_maps, list(range(NCORES)), trace=trace,
            tmpdir=os.environ.get("KERNEL_TRACE_DIR") or None)
    except Exception:
        if not trace:
            raise
        res = run_bass_kernel_spmd(nc, in_maps, list(range(NCORES)), trace=False)
    _cache["last_result"] = res
    parts = [res.results[c]["out"] for c in range(NCORES)]
    return combine_outputs(parts, (bv, Wo, bo))


# revision 24
# speedup vs baseline: 1.6880x; 1.0027x over previous
"""Multi-head attention (B=4, S=2048, D=1024, H=16) on 8 Trainium2 cores.

Sharding: core c handles batch b = c//2 and head-group hg = c%2 (8 of the 16
heads, 512 of the 1024 projection dims).  Host sums the two head-group
partials per batch (the "all-reduce after w_o") and adds bv@Wo + bo.

v2 design (vs the 425us baseline):
  * Pair-blocks: per (qc, pr) block both heads' scores are computed with
    64-row PE tiling (K=64 row tiles run concurrently -> 2x score matmul
    throughput; probe-measured 110ns/MM vs 216 standard).
  * Q is pre-scaled by log2(e)/8 on the host (folded into Wq/bq) so scores
    are in the exp2 domain.  exp for head-even tiles runs on ACT
    (exp(y*ln2)); head-odd tiles mostly run on the DVE as a one-instruction
    Schraudolph exp2 (tensor_scalar mult+add -> int16 bits == bf16 exp2
    approximation, ~3.3% max elementwise, cancels through softmax to ~1e-2
    final).  This splits the 293us exp load across two engines.
  * PSUM: SA + SB score tiles + two AV accumulators = exactly 8 banks.
    The l (softmax denominator) rides the AV matmul as V's 65th column.
  * Phase A emits K, V, Q(pair0) only; Q(pair 1-3) projections trail at
    qc0 block boundaries, C chunks of qc0 at qc1 boundaries, C of qc1 in
    the drain.  1/l uses reciprocal_approx_fast (5x the DVE reciprocal).

All matmuls are bf16 with fp32 PSUM accumulation.
"""

import os
import numpy as np

B, S, D = 4, 2048, 1024
H, DK = 16, 64
P = 128
NCORES = 8
HPC = H // 2            # heads per core
PROJ = HPC * DK         # 512 projection dims per core
NDM = D // P            # 8 d_model chunks
NPC = PROJ // P         # 4 head-pair chunks
NSC = S // 512          # 4 seq chunks of 512
NSO = S // P            # 16 seq chunks of 128
NKC = S // P            # 16 key chunks of 128

L2E = float(np.log2(np.e))
LN2 = float(np.log(2.0))
C_SCH = 16250.5
# Every kc runs one exp tile on ACT and one on the DVE (Schraudolph), so the
# two engines stream in parallel.  kc in DVE_KCS -> head-odd tile on DVE;
# else head-even on DVE.  Alternating keeps the approximation error split
# evenly across both heads.  f_schraudolph = 0.5 structurally.
DVE_KCS = frozenset(range(0, NKC, 2))

_cache = {}


def _build():
    import concourse.bass as bass
    import concourse.bacc as bacc
    import concourse.mybir as mybir
    import concourse.tile as tile
    from contextlib import ExitStack

    f32 = mybir.dt.float32
    bf16 = mybir.dt.bfloat16
    i16 = mybir.dt.int16
    AF = mybir.ActivationFunctionType
    MUL = mybir.AluOpType.mult
    ADD = mybir.AluOpType.add

    nc = bacc.Bacc("TRN2", target_bir_lowering=False, debug=False,
                   num_devices=NCORES)

    # Activations/weights arrive pre-permuted from the host so every DMA is
    # a dense per-partition burst (8KB lines) instead of 1KB strided lines.
    qT = nc.dram_tensor("qT", [NSC, P, NDM, 512], bf16, kind="ExternalInput").ap()
    kT = nc.dram_tensor("kT", [NSC, P, NDM, 512], bf16, kind="ExternalInput").ap()
    vT = nc.dram_tensor("vT", [NSC, P, NDM, 512], bf16, kind="ExternalInput").ap()
    wq = nc.dram_tensor("wq", [P, NDM, PROJ], bf16, kind="ExternalInput").ap()
    wk = nc.dram_tensor("wk", [P, NDM, PROJ], bf16, kind="ExternalInput").ap()
    wv = nc.dram_tensor("wv", [P, NDM, PROJ], bf16, kind="ExternalInput").ap()
    wo = nc.dram_tensor("wo", [P, NPC, D], bf16, kind="ExternalInput").ap()
    bq2 = nc.dram_tensor("bq2", [P, NPC], f32, kind="ExternalInput").ap()
    bk2 = nc.dram_tensor("bk2", [P, NPC], f32, kind="ExternalInput").ap()
    mk = nc.dram_tensor("mk", [P, NSO], f32, kind="ExternalInput").ap()
    sel = nc.dram_tensor("sel", [P, 128], bf16, kind="ExternalInput").ap()
    f16 = mybir.dt.float16
    out = nc.dram_tensor("out", [S, D], f16, kind="ExternalOutput").ap()

    with tile.TileContext(nc) as tc, ExitStack() as ctx:
        cpool = ctx.enter_context(tc.tile_pool(name="const", bufs=1))
        sel_sb = cpool.tile([P, 128], bf16)
        nc.sync.dma_start(sel_sb[:], sel)
        bq_sb = cpool.tile([P, NPC], f32)
        nc.sync.dma_start(bq_sb[:], bq2)
        bk_sb = cpool.tile([P, NPC], f32)
        nc.sync.dma_start(bk_sb[:], bk2)
        mk_sb = cpool.tile([P, NSO], f32)
        nc.sync.dma_start(mk_sb[:], mk)
        # l values land in rows {0, 32}; other rows must stay finite for the
        # sel broadcast matmul.
        Lsb = cpool.tile([P, 1024], bf16)
        nc.gpsimd.memset(Lsb[:], 0.0)
        ones8 = cpool.tile([P, HPC], bf16)
        nc.gpsimd.memset(ones8[:], 1.0)

        respool = ctx.enter_context(tc.tile_pool(name="res", bufs=1))
        # Q^T pair-stacked: rows 0-63 = head 2*pr dims, 64-127 = head
        # 2*pr+1.  K^T stored per head on the full 128-partition contraction
        # range (even heads rows 0-63, odd heads 64-127, rest zero) so the
        # score matmuls are full-array standard-mode matmuls: partial-array
        # tiling modes do not register as PE-busy in the HAM activity
        # monitor and leave the clock gate throttled at 1.2 GHz (measured:
        # 414us throttle-active with 64-row tiling vs 18us without).
        QT_sb = respool.tile([P, NPC, S], bf16)
        KT_sb = respool.tile([P, HPC, S], bf16)
        nc.vector.memset(KT_sb[:], 0.0)
        # V with an interleaved mask column per head: head h occupies cols
        # [h*65, h*65+64) and col h*65+64 == mask (the masked softmax
        # denominator rides the AV matmul as output partition 64).
        V_sb = respool.tile([P, NSO, HPC * (DK + 1)], bf16)
        for so in range(NSO):
            # mask columns filled on gpsimd so the DVE queue stays clear for
            # the K-projection bias adds.
            nc.gpsimd.tensor_scalar_mul(
                V_sb[:, so, :].rearrange("p (h w) -> p h w", w=DK + 1)[:, :, DK],
                ones8[:], mk_sb[:, so:so + 1])
        AT_sb = respool.tile([P, NPC, S], bf16)   # normalized A^T

        wopool = ctx.enter_context(tc.tile_pool(name="wo", bufs=1))
        wo_sb = wopool.tile([P, NPC, D], bf16)

        npool = ctx.enter_context(tc.tile_pool(name="norm", bufs=4))
        rcpool = ctx.enter_context(tc.tile_pool(name="rc", bufs=2))
        epA = ctx.enter_context(tc.tile_pool(name="expA", bufs=5))
        epB = ctx.enter_context(tc.tile_pool(name="expB", bufs=6))
        opool = ctx.enter_context(tc.tile_pool(name="ostage", bufs=4))

        # Weight + activation staging pools live for the whole kernel: wq is
        # needed for trailing Q projections inside phase B.
        wpool = ctx.enter_context(tc.tile_pool(name="w", bufs=2))
        apool = ctx.enter_context(tc.tile_pool(name="act", bufs=4))

        # ---------------- Phase A: K, V, Q(pair0) ----------------
        with ExitStack() as ctxA:
            psA = ctxA.enter_context(
                tc.tile_pool(name="psA", bufs=4, space="PSUM"))

            # K projection -> pair-packed KT_sb
            wk_sb = wpool.tile([P, NDM, PROJ], bf16, tag="w", name="wk_sb")
            nc.sync.dma_start(wk_sb[:], wk)
            for sc in range(NSC):
                a_sb = apool.tile([P, NDM, 512], bf16, tag="a", name="a_sb")
                nc.sync.dma_start(a_sb[:], kT[sc])
                for pc in range(NPC):
                    ps = psA.tile([P, 512], f32, tag="pp", name="psa")
                    for dc in range(NDM):
                        nc.tensor.matmul(
                            ps,
                            lhsT=wk_sb[:, dc, pc * P:(pc + 1) * P],
                            rhs=a_sb[:, dc, :],
                            start=(dc == 0), stop=(dc == NDM - 1))
                    for half in range(2):
                        lo = half * 64
                        nc.vector.tensor_scalar_add(
                            KT_sb[lo:lo + 64, 2 * pc + half,
                                  sc * 512:(sc + 1) * 512],
                            ps[lo:lo + 64, :],
                            bk_sb[lo:lo + 64, pc:pc + 1])

            # V projection (mask folded in)
            wv_sb = wpool.tile([P, NDM, PROJ], bf16, tag="w", name="wv_sb")
            nc.sync.dma_start(wv_sb[:], wv)
            for sc in range(NSC):
                a_sb = apool.tile([P, NDM, 512], bf16, tag="a", name="a_sb")
                nc.sync.dma_start(a_sb[:], vT[sc])
                for so4 in range(4):
                    so = sc * 4 + so4
                    ps = psA.tile([P, 512], f32, tag="pp", name="psa")
                    for dc in range(NDM):
                        nc.tensor.matmul(
                            ps,
                            lhsT=a_sb[:, dc, so4 * P:(so4 + 1) * P],
                            rhs=wv_sb[:, dc, :],
                            start=(dc == 0), stop=(dc == NDM - 1))
                    nc.vector.tensor_scalar_mul(
                        V_sb[:, so, :].rearrange(
                            "p (h w) -> p h w", w=DK + 1)[:, :, 0:DK],
                        ps.rearrange("p (h w) -> p h w", w=DK),
                        mk_sb[:, so:so + 1])

            # Q projection, pair 0 only; wq stays resident for pairs 1-3.
            wq_sb = wpool.tile([P, NDM, PROJ], bf16, tag="w", name="wq_sb")
            nc.sync.dma_start(wq_sb[:], wq)
            for sc in range(NSC):
                a_sb = apool.tile([P, NDM, 512], bf16, tag="a", name="a_sb")
                nc.sync.dma_start(a_sb[:], qT[sc])
                ps = psA.tile([P, 512], f32, tag="pp", name="psa")
                for dc in range(NDM):
                    nc.tensor.matmul(
                        ps,
                        lhsT=wq_sb[:, dc, 0:P],
                        rhs=a_sb[:, dc, :],
                        start=(dc == 0), stop=(dc == NDM - 1))
                nc.vector.tensor_scalar_add(
                    QT_sb[:, 0, sc * 512:(sc + 1) * 512], ps,
                    bq_sb[:, 0:1])
            nc.sync.dma_start(wo_sb[:], wo)

        # ---------------- Phase B ----------------
        # Four single-bank score pools: each 512-column half of each head's
        # score tile is its own PSUM tile, so the next kc's matmul into a
        # half only waits for the one exp instruction that read that half.
        psS4 = [ctx.enter_context(
            tc.tile_pool(name=f"psS{i}", bufs=1, space="PSUM"))
            for i in range(4)]
        psAcA = ctx.enter_context(tc.tile_pool(name="psAcA", bufs=1, space="PSUM"))
        psAcB = ctx.enter_context(tc.tile_pool(name="psAcB", bufs=1, space="PSUM"))

        qstage = {}

        def qprefetch(pc):
            """DMA the qT chunks for pair pc's trailing projection."""
            tiles = []
            for sc in range(NSC):
                a_sb = apool.tile([P, NDM, 512], bf16, tag="a", name="a_q")
                nc.sync.dma_start(a_sb[:], qT[sc])
                tiles.append(a_sb)
            qstage[pc] = tiles

        def emit_qproj(pc):
            """Trailing Q projection for pair pc (borrows psSA/psSB)."""
            tiles = qstage.pop(pc)
            for sc in range(NSC):
                a_sb = tiles[sc]
                pool = psS4[sc]
                ps = pool.tile([P, 512], f32, tag="s", name="ps_q")
                for dc in range(NDM):
                    nc.tensor.matmul(
                        ps,
                        lhsT=wq_sb[:, dc, pc * P:(pc + 1) * P],
                        rhs=a_sb[:, dc, :],
                        start=(dc == 0), stop=(dc == NDM - 1))
                nc.vector.tensor_scalar_add(
                    QT_sb[:, pc, sc * 512:(sc + 1) * 512], ps,
                    bq_sb[:, pc:pc + 1])

        def emit_c_chunk(so, oc, pool, dve_evac=False):
            """Output projection chunk [128q, 512o] (borrows a score pool)."""
            ps = pool.tile([P, 512], f32, tag="s", name="ps_c")
            for pc in range(NPC):
                nc.tensor.matmul(
                    ps,
                    lhsT=AT_sb[:, pc, so * P:(so + 1) * P],
                    rhs=wo_sb[:, pc, oc * 512:(oc + 1) * 512],
                    start=(pc == 0), stop=(pc == NPC - 1))
            ost = opool.tile([P, 512], f16, tag="o", name="ost")
            if dve_evac:
                nc.vector.tensor_copy(ost, ps)
            else:
                nc.scalar.copy(ost, ps)
            nc.sync.dma_start(
                out[so * P:(so + 1) * P, oc * 512:(oc + 1) * 512], ost)

        def emit_scores_exp(pr, qc, kc):
            """Standard-mode pair scores + the pair's exp, one tile/engine.

            Both engines consume their score halves as separate N=512
            instructions against separate single-bank PSUM tiles; the
            ACT-side tile is computed first since its chain is longest.
            """
            ea = epA.tile([P, 1024], bf16, tag="e", name="ea")
            eb = epB.tile([P, 1024], bf16, tag="e", name="eb")
            if kc in DVE_KCS:
                order = ((0, ea, True), (1, eb, False))
            else:
                order = ((1, eb, True), (0, ea, False))
            for hi, et, on_act in order:
                for sub in range(2):
                    ssl = slice(sub * 512, (sub + 1) * 512)
                    cols = slice(qc * 1024 + sub * 512,
                                 qc * 1024 + (sub + 1) * 512)
                    ps = psS4[2 * hi + sub].tile([P, 512], f32, tag="s",
                                                 name="ssc")
                    nc.tensor.matmul(
                        ps,
                        lhsT=KT_sb[:, 2 * pr + hi, kc * P:(kc + 1) * P],
                        rhs=QT_sb[:, pr, cols],
                        start=True, stop=True)
                    if on_act:
                        nc.scalar.activation(et[:, ssl], ps, AF.Exp,
                                             scale=LN2)
                    else:
                        nc.vector.tensor_scalar(
                            et[:, ssl].bitcast(i16), ps,
                            128.0, C_SCH, MUL, ADD)
            return ea, eb

        def emit_av(acc, h, e, kc, start, stop):
            for sub in range(2):
                nc.tensor.matmul(
                    acc[0:DK + 1, sub * 512:(sub + 1) * 512],
                    lhsT=V_sb[:, kc, h * (DK + 1):(h + 1) * (DK + 1)],
                    rhs=e[:, sub * 512:(sub + 1) * 512],
                    start=start, stop=stop)

        def tails_front(pr, qc, accA, accB, esA, esB):
            """Last AVs + PSUM evacuation (ACT does the big copies)."""
            emit_av(accA, 2 * pr + 0, esA[NKC - 2], NKC - 2, False, False)
            emit_av(accA, 2 * pr + 0, esA[NKC - 1], NKC - 1, False, True)
            emit_av(accB, 2 * pr + 1, esB[NKC - 2], NKC - 2, False, False)
            emit_av(accB, 2 * pr + 1, esB[NKC - 1], NKC - 1, False, True)
            nc.vector.tensor_copy(Lsb[0:1, :], accA[DK:DK + 1, :])
            nc.vector.tensor_copy(Lsb[32:33, :], accB[DK:DK + 1, :])
            atA = npool.tile([P, 1024], f32, tag="at", name="atA")
            nc.scalar.copy(atA[0:64, :], accA[0:64, :])
            atB = npool.tile([P, 1024], f32, tag="at", name="atB")
            nc.scalar.copy(atB[64:128, :], accB[0:64, :])
            return atA, atB

        def tails_back(pr, qc, atA, atB):
            """l broadcast (into accB's freed banks), 1/l, normalize.

            Runs after the boundary chunks so its serial ACT/DVE chain
            overlaps the chunks' matmuls; normalize on GPSIMD keeps the
            DVE free for the next block's exp stream.
            """
            bc = psAcB.tile([P, 1024], f32, tag="av", name="bc")
            for sub in range(2):
                nc.tensor.matmul(
                    bc[:, sub * 512:(sub + 1) * 512],
                    lhsT=sel_sb[:],
                    rhs=Lsb[:, sub * 512:(sub + 1) * 512],
                    start=True, stop=True)
            rc = rcpool.tile([P, 1024], f32, tag="rc", name="rc")
            nc.vector.reciprocal_approx_fast(out=rc[:], in_=bc[:])
            nc.gpsimd.tensor_mul(
                AT_sb[0:64, pr, qc * 1024:(qc + 1) * 1024],
                atA[0:64, :], rc[0:64, :])
            nc.gpsimd.tensor_mul(
                AT_sb[64:128, pr, qc * 1024:(qc + 1) * 1024],
                atB[64:128, :], rc[64:128, :])

        blocks = [(qc, pr) for qc in range(2) for pr in range(NPC)]
        for bi, (qc, pr) in enumerate(blocks):
            esA = [None] * NKC
            esB = [None] * NKC
            accA = accB = None
            for kc in range(NKC):
                esA[kc], esB[kc] = emit_scores_exp(pr, qc, kc)
                if kc == 6 and qc == 0 and pr < 3:
                    qprefetch(pr + 1)
                if kc == 2:
                    accA = psAcA.tile([P, 1024], f32, tag="av", name="accA")
                    accB = psAcB.tile([P, 1024], f32, tag="av", name="accB")
                    emit_av(accA, 2 * pr + 0, esA[0], 0, True, False)
                    emit_av(accB, 2 * pr + 1, esB[0], 0, True, False)
                elif kc > 2:
                    emit_av(accA, 2 * pr + 0, esA[kc - 2], kc - 2, False, False)
                    emit_av(accB, 2 * pr + 1, esB[kc - 2], kc - 2, False, False)
            atA, atB = tails_front(pr, qc, accA, accB, esA, esB)

            # boundary work between the evacuation and the bc/rc/normalize
            # chain, so the PE streams chunks while ACT/DVE drain the
            # accumulators: trailing Q projections during qc0, qc0's output
            # projection during qc1 boundaries; qc1's C lands in the drain.
            # bi==3 must normalize first (its own AT feeds the chunks) and
            # bi==7 is the drain, handled below.
            tc.no_sync_barrier()
            if qc == 0 and pr < 3:
                emit_qproj(pr + 1)
                tails_back(pr, qc, atA, atB)
            elif bi == 3:
                # this block's own AT feeds the chunks: normalize the first
                # q-half on the DVE (critical path), release the chunks that
                # only read it, finish the second half on gpsimd behind them.
                bc = psAcB.tile([P, 1024], f32, tag="av", name="bc")
                for sub in range(2):
                    nc.tensor.matmul(
                        bc[:, sub * 512:(sub + 1) * 512],
                        lhsT=sel_sb[:],
                        rhs=Lsb[:, sub * 512:(sub + 1) * 512],
                        start=True, stop=True)
                rc = rcpool.tile([P, 1024], f32, tag="rc", name="rc")
                nc.vector.reciprocal_approx_fast(out=rc[:], in_=bc[:])
                nc.vector.tensor_tensor(
                    AT_sb[0:64, pr, 0:512], atA[0:64, 0:512],
                    rc[0:64, 0:512], MUL)
                nc.vector.tensor_tensor(
                    AT_sb[64:128, pr, 0:512], atB[64:128, 0:512],
                    rc[64:128, 0:512], MUL)
                tc.no_sync_barrier()
                for j in range(4):
                    emit_c_chunk(j // 2, j % 2, psS4[j % 4])
                nc.gpsimd.tensor_mul(
                    AT_sb[0:64, pr, 512:1024], atA[0:64, 512:1024],
                    rc[0:64, 512:1024])
                nc.gpsimd.tensor_mul(
                    AT_sb[64:128, pr, 512:1024], atB[64:128, 512:1024],
                    rc[64:128, 512:1024])
            elif bi < 7:
                cidx = (bi - 3) * 4
                hi = cidx + (6 if bi == 6 else 4)
                for j in range(cidx, min(hi, 14)):
                    emit_c_chunk(j // 2, j % 2, psS4[j % 4])
                tails_back(pr, qc, atA, atB)

        # drain: two reserved qc0 chunks keep the PE busy while the last
        # block's accumulators are evacuated, then the final normalize runs
        # in quarters (on the DVE -- it is on the critical path here), each
        # quarter releasing 4 output chunks of qc1's projection.
        tc.no_sync_barrier()
        emit_c_chunk(7, 0, psS4[0])
        emit_c_chunk(7, 1, psS4[1])
        bc = psAcB.tile([P, 1024], f32, tag="av", name="bc")
        for sub in range(2):
            nc.tensor.matmul(
                bc[:, sub * 512:(sub + 1) * 512],
                lhsT=sel_sb[:],
                rhs=Lsb[:, sub * 512:(sub + 1) * 512],
                start=True, stop=True)
        rc = rcpool.tile([P, 1024], f32, tag="rc", name="rc")
        nc.vector.reciprocal_approx_fast(out=rc[:], in_=bc[:])
        for qt in range(4):
            hsl = slice(qt * 256, (qt + 1) * 256)
            nc.vector.tensor_tensor(
                AT_sb[0:64, 3, 1024 + qt * 256:1024 + (qt + 1) * 256],
                atA[0:64, hsl], rc[0:64, hsl], MUL)
            nc.vector.tensor_tensor(
                AT_sb[64:128, 3, 1024 + qt * 256:1024 + (qt + 1) * 256],
                atB[64:128, hsl], rc[64:128, hsl], MUL)
            tc.no_sync_barrier()
            for j in range(4):
                so, oc = 8 + qt * 2 + j // 2, j % 2
                emit_c_chunk(so, oc, psS4[j % 4], dve_evac=(j % 2 == 1))

    nc.compile()
    return nc


def _get_nc():
    if "nc" not in _cache:
        _cache["nc"] = _build()
    return _cache["nc"]


def make_in_maps(q, k, v, mask, Wq, bq, Wk, bk, Wv, bv, Wo, bo):
    """Host-side sharding: slice/transpose the full inputs per core."""
    import ml_dtypes
    f = np.float32
    bf = ml_dtypes.bfloat16
    q = np.asarray(q, dtype=f)
    k = np.asarray(k, dtype=f)
    v = np.asarray(v, dtype=f)
    Wq = np.asarray(Wq, dtype=f) * (L2E / 8.0)   # exp2-domain prescale
    Wk = np.asarray(Wk, dtype=f)
    Wv = np.asarray(Wv, dtype=f)
    Wo = np.asarray(Wo, dtype=f)
    bq = np.asarray(bq, dtype=f) * (L2E / 8.0)
    bk = np.asarray(bk, dtype=f)
    mask = np.asarray(mask)

    sel = np.zeros((P, 128), dtype=f)
    sel[0, 0:64] = 1.0
    sel[32, 64:128] = 1.0

    def act_perm(x):
        # [S, D] -> [NSC, P, NDM, 512]: chunk sc holds x^T[d, sc*512 + j]
        # with d = o*128 + p, laid out as one dense 8KB line per partition.
        return np.ascontiguousarray(
            x.reshape(NSC, 512, NDM, P).transpose(0, 3, 2, 1)).astype(bf)

    def w_perm(w):
        # [D, PROJ] -> [P, NDM, PROJ]
        return np.ascontiguousarray(
            w.reshape(NDM, P, -1).transpose(1, 0, 2)).astype(bf)

    in_maps = []
    for c in range(NCORES):
        b, hg = divmod(c, 2)
        cols = slice(hg * PROJ, (hg + 1) * PROJ)
        mvals = (mask[b, 0, 0, :] != 0).astype(f)            # [S]
        mk2 = np.ascontiguousarray(mvals.reshape(NSO, P).T)  # [P, NSO]
        in_maps.append({
            "qT": act_perm(q[b]),
            "kT": act_perm(k[b]),
            "vT": act_perm(v[b]),
            "wq": w_perm(Wq[:, cols]),
            "wk": w_perm(Wk[:, cols]),
            "wv": w_perm(Wv[:, cols]),
            "wo": np.ascontiguousarray(
                Wo[cols, :].reshape(NPC, P, D).transpose(1, 0, 2)).astype(bf),
            "bq2": np.ascontiguousarray(bq[cols].reshape(NPC, P).T),
            "bk2": np.ascontiguousarray(bk[cols].reshape(NPC, P).T),
            "mk": mk2,
            "sel": sel.astype(bf),
        })
    return in_maps


def combine_outputs(parts, Wv_bv_Wo_bo):
    """Sum the two head-group partials per batch, add bv @ Wo + bo."""
    bv, Wo, bo = Wv_bv_Wo_bo
    bo_eff = (np.asarray(bv, np.float32) @ np.asarray(Wo, np.float32)
              + np.asarray(bo, np.float32))
    out = np.empty((B, S, D), dtype=np.float32)
    for b in range(B):
        out[b] = parts[2 * b] + parts[2 * b + 1] + bo_eff
    return out


def _install_axon_ntff_hook():
    """The agent image's antenv lacks axon_hooks; synthesize it and register
    the ctypes NTFF profile hook from trn_boot so trace=True works."""
    import sys
    import types
    if "antenv.axon_hooks" in sys.modules:
        return
    try:
        from trn_agent_boot.trn_boot import _ntff_profile_via_ctypes
        hook = _ntff_profile_via_ctypes("/opt/axon/libaxon_pjrt.so")
    except Exception:
        hook = None
    mod = types.ModuleType("antenv.axon_hooks")
    mod._hook = hook
    mod.get_axon_ntff_profile_hook = lambda: mod._hook
    mod.set_axon_ntff_profile_hook = lambda h: setattr(mod, "_hook", h)
    sys.modules["antenv.axon_hooks"] = mod
    import concourse.bass_utils as bu
    bu.upload_artifacts = lambda tmpdir: str(tmpdir)


def kernel(q, k, v, mask, Wq, bq, Wk, bk, Wv, bv, Wo, bo):
    from concourse.bass_utils import run_bass_kernel_spmd

    nc = _get_nc()
    in_maps = make_in_maps(q, k, v, mask, Wq, bq, Wk, bk, Wv, bv, Wo, bo)
    trace = bool(int(os.environ.get("KERNEL_TRACE", "0")))
    if trace:
        try:
            _install_axon_ntff_hook()
        except Exception:
            trace = False
    try:
        res = run_bass_kernel_spmd(
            nc, in_maps, list(range(NCORES)), trace=trace,
            tmpdir=os.environ.get("KERNEL_TRACE_DIR") or None)
    except Exception:
        if not trace:
            raise
        res = run_bass_kernel_spmd(nc, in_maps, list(range(NCORES)), trace=False)
    _cache["last_result"] = res
    parts = [res.results[c]["out"] for c in range(NCORES)]
    return combine_outputs(parts, (bv, Wo, bo))
